# revision 29
# baseline (speedup 1.0000x reference)
"""Bass/Trainium2 kernel for nn_ACModel (NTM-style content addressing + LSTM + actor/critic).

Sharding (8 cores):
 - feat / c_tm1: data-parallel over batch (16 samples per core).
 - x / W_x2cont: k-split over the 10000-dim contraction (1250 per core) computed
   full-batch, combined with an AllReduce.
 - W_actor / b_actor: w-split over the 10000 output words (1250 per core), with an
   AllGather of the actor hidden layer; each core emits logits[:, its slice].
 - everything else replicated.
kernel(**inputs) takes FULL unsharded numpy inputs and returns (action_logit,
value, h, c) exactly like the reference.
"""

import numpy as np
from contextlib import ExitStack

import concourse.bass as bass
import concourse.bacc as bacc
import concourse.tile as tile
from concourse import mybir
from concourse.bass_utils import run_bass_kernel_spmd
from concourse.masks import make_identity

F32 = mybir.dt.float32
BF16 = mybir.dt.bfloat16
I32 = mybir.dt.int32
AF = mybir.ActivationFunctionType
ALU = mybir.AluOpType

# ---- problem shapes (hardcoded) ----
B, S, WN = 128, 4096, 10000
D = 256          # FEAT
CONT = 256
LSTM = 256
AH, CH = 128, 128
NCORES = 8
BL = B // NCORES          # 16 samples per core
NT = S // 128             # 32 slot-tiles of 128 per sample
HT = NT // 2
WSL = WN // NCORES        # 1250: per-core slice of the 10000 dim
# chunking of the per-core 1250-wide slice: 9 x 128 + 98
KCHUNKS = [(k * 128, 128) for k in range(9)] + [(1152, 98)]
# actor logit segments within the per-core 1250 columns (each <=512 psum bank)
ASEGS = [(0, 500), (500, 500), (1000, 250)]
# norm^2 tiles computed on DVE instead of ACT (load balancing)
DVE_NORM_TILES = (2, 7, 12, 17, 22, 27)
RSQRT_MAGIC = 0x5F3759DF
REPL = [list(range(NCORES))]


def _bcast_ap(src_ap: bass.AP, parts: int) -> bass.AP:
    return bass.AP(
        tensor=src_ap.tensor,
        offset=src_ap.offset,
        ap=[[0, parts]] + [list(p) for p in src_ap.ap],
    )


def build_nc(phase_limit: int = 3) -> bass.Bass:
    nc = bacc.Bacc()

    # ---------------- dram parameters ----------------
    xs_d = nc.declare_dram_parameter("x_slice", [B, WSL], F32, isOutput=False)
    feat_d = nc.declare_dram_parameter("feat", [BL, S, D], F32, isOutput=False)
    h_d = nc.declare_dram_parameter("h_tm1", [B, LSTM], F32, isOutput=False)
    c_d = nc.declare_dram_parameter("c_tm1", [BL, LSTM], F32, isOutput=False)
    sel_d = nc.declare_dram_parameter("sel", [B, BL], F32, isOutput=False)
    Wx2cs_d = nc.declare_dram_parameter("W_x2cont_slice", [CONT, WSL], F32, isOutput=False)
    bx2c_d = nc.declare_dram_parameter("b_x2cont", [CONT], F32, isOutput=False)
    Wh2c_d = nc.declare_dram_parameter("W_h2cont", [CONT, LSTM], F32, isOutput=False)
    bh2c_d = nc.declare_dram_parameter("b_h2cont", [CONT], F32, isOutput=False)
    Wc2k_d = nc.declare_dram_parameter("W_cont2key", [D, CONT], F32, isOutput=False)
    bc2k_d = nc.declare_dram_parameter("b_cont2key", [D], F32, isOutput=False)
    Wc2s_d = nc.declare_dram_parameter("W_cont2strength", [1, CONT], F32, isOutput=False)
    bc2s_d = nc.declare_dram_parameter("b_cont2strength", [1], F32, isOutput=False)
    Watt_d = nc.declare_dram_parameter("W_att_gates", [1, D + LSTM + CONT], F32, isOutput=False)
    batt_d = nc.declare_dram_parameter("b_att_gates", [1], F32, isOutput=False)
    Wih_d = nc.declare_dram_parameter("W_ih", [4 * LSTM, D + CONT], F32, isOutput=False)
    bih_d = nc.declare_dram_parameter("b_ih", [4 * LSTM], F32, isOutput=False)
    Whh_d = nc.declare_dram_parameter("W_hh", [4 * LSTM, LSTM], F32, isOutput=False)
    bhh_d = nc.declare_dram_parameter("b_hh", [4 * LSTM], F32, isOutput=False)
    Wah_d = nc.declare_dram_parameter("W_actor_hid", [AH, LSTM], F32, isOutput=False)
    bah_d = nc.declare_dram_parameter("b_actor_hid", [AH], F32, isOutput=False)
    Wch_d = nc.declare_dram_parameter("W_critic_hid", [CH, LSTM], F32, isOutput=False)
    bch_d = nc.declare_dram_parameter("b_critic_hid", [CH], F32, isOutput=False)
    Wacts_d = nc.declare_dram_parameter("W_actor_slice", [WSL, AH], F32, isOutput=False)
    bacts_d = nc.declare_dram_parameter("b_actor_slice", [WSL], F32, isOutput=False)
    Wcrit_d = nc.declare_dram_parameter("W_critic", [1, CH], F32, isOutput=False)
    bcrit_d = nc.declare_dram_parameter("b_critic", [1], F32, isOutput=False)

    out_logit = nc.declare_dram_parameter("action_logit", [B, WSL], F32, isOutput=True)
    out_value = nc.declare_dram_parameter("value", [BL, 1], F32, isOutput=True)
    out_h = nc.declare_dram_parameter("h", [BL, LSTM], F32, isOutput=True)
    out_c = nc.declare_dram_parameter("c", [BL, LSTM], F32, isOutput=True)

    with ExitStack() as ctx:
        tc = ctx.enter_context(tile.TileContext(nc))
        singles = ctx.enter_context(tc.tile_pool(name="singles", bufs=1))
        ptr = ctx.enter_context(tc.tile_pool(name="ptr", bufs=3, space="PSUM"))
        dramp = ctx.enter_context(tc.tile_pool(name="dramp", bufs=1, space="DRAM"))

        # ---------------- constants / staging ----------------
        identity = singles.tile([128, 128], F32, tag="identity")
        make_identity(nc, identity)
        ones_col = singles.tile([128, 1], F32, tag="ones_col")
        nc.vector.memset(ones_col, 1.0)
        ones_row = singles.tile([1, 128], F32, tag="ones_row")
        nc.vector.memset(ones_row, 1.0)

        h_sb = singles.tile([B, LSTM], F32, tag="h_sb")
        nc.sync.dma_start(out=h_sb, in_=h_d[:, :])
        c_sb = singles.tile([BL, LSTM], F32, tag="c_sb")
        nc.sync.dma_start(out=c_sb, in_=c_d[:, :])
        sel_sb = singles.tile([B, BL], F32, tag="sel_sb")
        nc.sync.dma_start(out=sel_sb, in_=sel_d[:, :])
        x_sb = singles.tile([B, WSL], F32, tag="x_sb")
        nc.sync.dma_start(out=x_sb, in_=xs_d[:, :])

        def bcast_tile(name, src_row_ap, parts, width):
            t = singles.tile([parts, width], F32, tag=name)
            nc.sync.dma_start(out=t, in_=_bcast_ap(src_row_ap, parts))
            return t

        bx2c_rep = bcast_tile("bx2c_rep", bx2c_d[:], B, CONT)
        bh2c_rep = bcast_tile("bh2c_rep", bh2c_d[:], B, CONT)
        bc2k_rep = bcast_tile("bc2k_rep", bc2k_d[:], B, D)
        wstr_rep = bcast_tile("wstr_rep", Wc2s_d[0, :], B, CONT)
        bc2s_rep = bcast_tile("bc2s_rep", bc2s_d[:], B, 1)
        watt_rep = bcast_tile("watt_rep", Watt_d[0, :], BL, D + LSTM + CONT)
        batt_rep = bcast_tile("batt_rep", batt_d[:], BL, 1)
        bih_rep = bcast_tile("bih_rep", bih_d[:], BL, 4 * LSTM)
        bhh_rep = bcast_tile("bhh_rep", bhh_d[:], BL, 4 * LSTM)
        bah_rep = bcast_tile("bah_rep", bah_d[:], BL, AH)
        bch_rep = bcast_tile("bch_rep", bch_d[:], BL, CH)
        wcrit_rep = bcast_tile("wcrit_rep", Wcrit_d[0, :], BL, CH)
        bcrit_rep = bcast_tile("bcrit_rep", bcrit_d[:], BL, 1)
        bact_row = singles.tile([1, WSL], F32, tag="bact_row")
        nc.sync.dma_start(out=bact_row, in_=bacts_d[:].rearrange("(a b) -> a b", a=1))

        # ---------------- helpers ----------------
        tr_count = [0]

        def pe_transpose(dst_ap, src_ap):
            p, f = src_ap.shape[0], src_ap.shape[1]
            pt = ptr.tile([128, 128], F32, tag="tr_ps")
            nc.tensor.transpose(pt[:f, :p], src_ap, identity[:p, :p])
            if tr_count[0] % 2 == 0:
                nc.vector.tensor_copy(dst_ap, pt[:f, :p])
            else:
                nc.scalar.copy(dst_ap, pt[:f, :p])
            tr_count[0] += 1

        def dve_rsqrt(dst, z, pool, tag, parts, width):
            """dst = 1/sqrt(z) elementwise via bit-hack + 2 Newton steps (DVE only)."""
            zb = z.bitcast(I32)
            sh = pool.tile([parts, width], I32, tag=tag + "_sh")
            nc.vector.tensor_scalar(out=sh, in0=zb, scalar1=1, scalar2=None,
                                    op0=ALU.logical_shift_right)
            nt = pool.tile([parts, width], I32, tag=tag + "_nt")
            nc.vector.tensor_scalar(out=nt, in0=sh, scalar1=0, scalar2=None,
                                    op0=ALU.bitwise_not)
            nc.vector.tensor_scalar(out=nt, in0=nt, scalar1=RSQRT_MAGIC + 1,
                                    scalar2=None, op0=ALU.add)
            y = nt.bitcast(F32)
            t = pool.tile([parts, width], F32, tag=tag + "_t")
            for it in range(2):
                nc.vector.tensor_mul(t, y, y)
                nc.vector.tensor_mul(t, t, z)
                nc.vector.tensor_scalar(out=t, in0=t, scalar1=-0.5, scalar2=1.5,
                                        op0=ALU.mult, op1=ALU.add)
                if it == 1:
                    nc.vector.tensor_mul(dst, y, t)
                else:
                    nc.vector.tensor_mul(y, y, t)

        # ---------------- one-time PE weight transposes ----------------
        # x_slice^T chunks: xT[:wsz, k, :] = x_slice[:, ko:ko+wsz]^T
        xT_sb = singles.tile([128, len(KCHUNKS), B], BF16, tag="xT_sb")
        for k, (ko, wsz) in enumerate(KCHUNKS):
            pe_transpose(xT_sb[:wsz, k, :], x_sb[:, ko:ko + wsz])

        # h^T (full batch, f32) for h_cont
        hT_sb = singles.tile([128, 2, B], F32, tag="hT_sb")
        for lc in range(2):
            pe_transpose(hT_sb[:, lc, :], h_sb[:, lc * 128:(lc + 1) * 128])

        # W_h2cont^T / W_cont2key^T  [128, lc, 256]
        Wh2c_st = singles.tile([128, 2, LSTM], F32, tag="Wh2c_st")
        Wc2k_st = singles.tile([128, 2, CONT], F32, tag="Wc2k_st")
        Wh2cT_sb = singles.tile([128, 2, CONT], F32, tag="Wh2cT_sb")
        Wc2kT_sb = singles.tile([128, 2, D], F32, tag="Wc2kT_sb")
        for mh in range(2):
            nc.sync.dma_start(out=Wh2c_st[:, mh, :], in_=Wh2c_d[mh * 128:(mh + 1) * 128, :])
            nc.sync.dma_start(out=Wc2k_st[:, mh, :], in_=Wc2k_d[mh * 128:(mh + 1) * 128, :])
        for mh in range(2):
            for lc in range(2):
                pe_transpose(Wh2cT_sb[:, lc, mh * 128:(mh + 1) * 128],
                             Wh2c_st[:, mh, lc * 128:(lc + 1) * 128])
                pe_transpose(Wc2kT_sb[:, lc, mh * 128:(mh + 1) * 128],
                             Wc2k_st[:, mh, lc * 128:(lc + 1) * 128])

        # W_x2cont_slice^T chunks
        wstage = ctx.enter_context(tc.tile_pool(name="wstage", bufs=4))
        wxp = ctx.enter_context(tc.tile_pool(name="wxp", bufs=2))
        Wx2cT_sb = singles.tile([128, len(KCHUNKS), CONT], BF16, tag="Wx2cT_sb")
        for mh in range(2):
            st = wxp.tile([128, WSL], F32, tag="wx_st")
            nc.sync.dma_start(out=st, in_=Wx2cs_d[mh * 128:(mh + 1) * 128, :])
            for k, (ko, wsz) in enumerate(KCHUNKS):
                pe_transpose(Wx2cT_sb[:wsz, k, mh * 128:(mh + 1) * 128],
                             st[:, ko:ko + wsz])

        # ---------------- phase 1: controller (full batch of 128) ----------------
        xcont_sb = singles.tile([B, CONT], F32, tag="xcont_sb")
        ctrl_sb = singles.tile([B, CONT], F32, tag="ctrl_sb")
        keyv_sb = singles.tile([B, D], F32, tag="keyv_sb")
        keyv_loc = singles.tile([BL, D], F32, tag="keyv_loc")
        xcont_loc = singles.tile([BL, CONT], F32, tag="xcont_loc")
        h_loc = singles.tile([BL, LSTM], F32, tag="h_loc")
        cb_all = singles.tile([128, BL], F32, tag="cb_all")
        ns_all = singles.tile([128, BL], F32, tag="ns_all")
        scal_loc = singles.tile([BL, 2], F32, tag="scal_loc")

        with tc.tile_pool(name="pctrl", bufs=2, space="PSUM") as pctrl:
            # partial x_cont over this core's slice, full batch, then AllReduce
            ps_xc = pctrl.tile([B, CONT], F32, tag="ps_ctrl")
            for k, (ko, wsz) in enumerate(KCHUNKS):
                nc.tensor.matmul(ps_xc, lhsT=xT_sb[:wsz, k, :], rhs=Wx2cT_sb[:wsz, k, :],
                                 start=(k == 0), stop=(k == len(KCHUNKS) - 1))
            xc_part = singles.tile([B, CONT], F32, tag="xc_part")
            nc.vector.tensor_copy(xc_part, ps_xc)
            cc_in = dramp.tile([B, CONT], F32, tag="cc_in")
            nc.sync.dma_start(out=cc_in, in_=xc_part)
            cc_out = dramp.tile([B, CONT], F32, tag="cc_out", addr_space="Shared")
            nc.gpsimd.collective_compute(
                "AllReduce", ALU.add, replica_groups=REPL,
                ins=[cc_in[:, :]], outs=[cc_out[:, :]])
            xc_full = singles.tile([B, CONT], F32, tag="xc_full")
            nc.sync.dma_start(out=xc_full, in_=cc_out)
            nc.vector.tensor_add(xcont_sb, xc_full, bx2c_rep)

            # h_cont + control
            ps_hc = pctrl.tile([B, CONT], F32, tag="ps_ctrl")
            for lc in range(2):
                nc.tensor.matmul(ps_hc, lhsT=hT_sb[:, lc, :], rhs=Wh2cT_sb[:, lc, :],
                                 start=(lc == 0), stop=(lc == 1))
            t1 = singles.tile([B, CONT], F32, tag="ph1_t1")
            nc.vector.tensor_add(t1, ps_hc, xcont_sb)
            nc.vector.tensor_add(t1, t1, bh2c_rep)
            nc.vector.tensor_scalar_max(ctrl_sb, t1, 0.0)

            ctrlT_sb = singles.tile([128, 2, B], F32, tag="ctrlT_sb")
            for lc in range(2):
                pe_transpose(ctrlT_sb[:, lc, :], ctrl_sb[:, lc * 128:(lc + 1) * 128])

            # key_v = tanh(control @ Wc2k^T + b)
            ps_kv = pctrl.tile([B, D], F32, tag="ps_ctrl")
            for lc in range(2):
                nc.tensor.matmul(ps_kv, lhsT=ctrlT_sb[:, lc, :], rhs=Wc2kT_sb[:, lc, :],
                                 start=(lc == 0), stop=(lc == 1))
            kv_pre = singles.tile([B, D], F32, tag="kv_pre")
            nc.vector.tensor_add(kv_pre, ps_kv, bc2k_rep)
            nc.scalar.activation(keyv_sb, kv_pre, AF.Tanh)

            # strength, c = strength * rsqrt(||key||^2), negstr
            str_scr = singles.tile([B, CONT], F32, tag="str_scr")
            str_pre = singles.tile([B, 1], F32, tag="str_pre")
            nc.vector.scalar_tensor_tensor(out=str_scr, in0=ctrl_sb, scalar=1.0,
                                           in1=wstr_rep, op0=ALU.mult, op1=ALU.mult,
                                           accum_out=str_pre)
            strength = singles.tile([B, 1], F32, tag="strength")
            nc.vector.tensor_add(strength, str_pre, bc2s_rep)
            nc.vector.tensor_scalar_max(strength, strength, 0.0)
            nc.vector.tensor_scalar_add(strength, strength, 1.0)
            kn_scr = singles.tile([B, D], F32, tag="kn_scr")
            kn2 = singles.tile([B, 1], F32, tag="kn2")
            nc.scalar.activation(kn_scr, keyv_sb, AF.Square, accum_out=kn2)
            kn_rs = singles.tile([B, 1], F32, tag="kn_rs")
            dve_rsqrt(kn_rs, kn2, singles, "knrs", B, 1)
            pack = singles.tile([B, 2], F32, tag="pack")
            nc.vector.tensor_mul(pack[:, 0:1], strength, kn_rs)
            nc.vector.tensor_scalar_mul(pack[:, 1:2], strength, -1.0)

            # extract local rows: loc = sel^T @ full
            def extract(dst, src, width):
                ps = pctrl.tile([BL, CONT], F32, tag="ps_ext")
                nc.tensor.matmul(ps[:, :width], lhsT=sel_sb, rhs=src,
                                 start=True, stop=True)
                nc.vector.tensor_copy(dst, ps[:, :width])

            extract(keyv_loc, keyv_sb, D)
            extract(xcont_loc, xcont_sb, CONT)
            extract(h_loc, h_sb, LSTM)
            extract(scal_loc, pack, 2)

            # per-sample exp scale/bias broadcast across partitions
            for col, dst, tg in ((0, cb_all, "cbr"), (1, ns_all, "nsr")):
                ptc = ptr.tile([128, 128], F32, tag="tr_ps")
                nc.tensor.transpose(ptc[:1, :BL], scal_loc[:, col:col + 1],
                                    identity[:BL, :BL])
                row = singles.tile([1, BL], F32, tag=tg)
                nc.vector.tensor_copy(row, ptc[:1, :BL])
                nc.gpsimd.partition_broadcast(dst, row)

        # W_ih^T / W_hh^T (bf16, phase 3)
        WihT_sb = singles.tile([128, 4, 4 * LSTM], BF16, tag="WihT_sb")
        for gc in range(8):
            st = wstage.tile([128, D + CONT], F32, tag="wih_st")
            nc.sync.dma_start(out=st, in_=Wih_d[gc * 128:(gc + 1) * 128, :])
            for kc in range(4):
                pe_transpose(WihT_sb[:, kc, gc * 128:(gc + 1) * 128],
                             st[:, kc * 128:(kc + 1) * 128])
        WhhT_sb = singles.tile([128, 2, 4 * LSTM], BF16, tag="WhhT_sb")
        for gc in range(8):
            st = wstage.tile([128, LSTM], F32, tag="whh_st")
            nc.sync.dma_start(out=st, in_=Whh_d[gc * 128:(gc + 1) * 128, :])
            for lc in range(2):
                pe_transpose(WhhT_sb[:, lc, gc * 128:(gc + 1) * 128],
                             st[:, lc * 128:(lc + 1) * 128])
        WahT_sb = singles.tile([128, 2, AH], BF16, tag="WahT_sb")
        WchT_sb = singles.tile([128, 2, CH], BF16, tag="WchT_sb")
        st_ah = singles.tile([AH, LSTM], F32, tag="st_ah")
        nc.sync.dma_start(out=st_ah, in_=Wah_d[:, :])
        st_ch = singles.tile([CH, LSTM], F32, tag="st_ch")
        nc.sync.dma_start(out=st_ch, in_=Wch_d[:, :])
        for lc in range(2):
            pe_transpose(WahT_sb[:, lc, :], st_ah[:, lc * 128:(lc + 1) * 128])
            pe_transpose(WchT_sb[:, lc, :], st_ch[:, lc * 128:(lc + 1) * 128])
        # W_actor_slice^T (bf16) [128 ah, 1250]
        WactT_sb = singles.tile([128, WSL], BF16, tag="WactT_sb")
        for k, (ko, wsz) in enumerate(KCHUNKS):
            st = wstage.tile([128, AH], F32, tag="wact_st")
            nc.sync.dma_start(out=st[:wsz, :], in_=Wacts_d[ko:ko + wsz, :])
            pe_transpose(WactT_sb[:, ko:ko + wsz], st[:wsz, :])


        # ---------------- phase 2: memory read (per sample streaming) ----------------
        catt_all = singles.tile([BL, D], F32, tag="catt_all")

        featp = ctx.enter_context(tc.tile_pool(name="featp", bufs=4))
        keyrp = ctx.enter_context(tc.tile_pool(name="keyrp", bufs=3))
        scrp = ctx.enter_context(tc.tile_pool(name="scrp", bufs=2))
        smallp = ctx.enter_context(tc.tile_pool(name="smallp", bufs=4))
        tinyp = ctx.enter_context(tc.tile_pool(name="tinyp", bufs=2))
        with tc.tile_pool(name="pcand", bufs=2, space="PSUM") as pcand:
            for b in (range(BL) if phase_limit >= 2 else []):
                key_row = keyrp.tile([1, D], F32, tag="key_row")
                nc.sync.dma_start(out=key_row, in_=keyv_loc[b:b + 1, :])
                key_rep = keyrp.tile([128, D], F32, tag="key_rep")
                nc.gpsimd.partition_broadcast(key_rep, key_row)
                cb = cb_all[:, b:b + 1]
                nsb = ns_all[:, b:b + 1]

                fsrc = feat_d[b].rearrange("(p j) d -> p j d", p=128)
                fhalves = []
                for hf in range(2):
                    fh = featp.tile([128, HT, D], F32, tag="fbh")
                    for g in range(2):
                        go = hf * HT + g * 8
                        nc.sync.dma_start(out=fh[:, g * 8:(g + 1) * 8, :],
                                          in_=fsrc[:, go:go + 8, :])
                    fhalves.append(fh)

                def fbt(t):
                    return fhalves[t // HT][:, t % HT, :]

                ps_cd = pcand.tile([1, D + 1], F32, tag="ps_cd")
                for hf in range(2):
                    dots = smallp.tile([128, HT], F32, tag="dots")
                    nrm2 = smallp.tile([128, HT], F32, tag="nrm2")
                    for tl in range(HT):
                        t = hf * HT + tl
                        scr = scrp.tile([128, D], F32, tag="scr_dve")
                        nc.vector.scalar_tensor_tensor(
                            out=scr, in0=fbt(t), scalar=1.0, in1=key_rep,
                            op0=ALU.mult, op1=ALU.mult, accum_out=dots[:, tl:tl + 1])
                        if t in DVE_NORM_TILES:
                            scr2 = scrp.tile([128, D], F32, tag="scr_dve2")
                            nc.vector.scalar_tensor_tensor(
                                out=scr2, in0=fbt(t), scalar=1.0, in1=fbt(t),
                                op0=ALU.mult, op1=ALU.mult,
                                accum_out=nrm2[:, tl:tl + 1])
                        else:
                            scr3 = scrp.tile([128, D], F32, tag="sq_scr")
                            nc.scalar.activation(scr3, fbt(t), AF.Square,
                                                 accum_out=nrm2[:, tl:tl + 1])

                    rs = smallp.tile([128, HT], F32, tag="rs")
                    dve_rsqrt(rs, nrm2, smallp, "p2rs", 128, HT)
                    u = smallp.tile([128, HT], F32, tag="u")
                    nc.vector.tensor_mul(u, dots, rs)
                    e = smallp.tile([128, HT], F32, tag="e")
                    if hf == 0:
                        esum_all = smallp.tile([128, 2], F32, tag="esum_all")
                    nc.scalar.activation(e, u, AF.Exp, scale=cb, bias=nsb,
                                         accum_out=esum_all[:, hf:hf + 1])

                    for tl in range(HT):
                        t = hf * HT + tl
                        nc.tensor.matmul(ps_cd[:, 0:D], lhsT=e[:, tl:tl + 1],
                                         rhs=fbt(t), start=(t == 0), stop=(t == NT - 1))
                esum_tot = smallp.tile([128, 1], F32, tag="esum_tot")
                nc.vector.tensor_reduce(out=esum_tot, in_=esum_all,
                                        axis=mybir.AxisListType.X, op=ALU.add)
                nc.tensor.matmul(ps_cd[:, D:D + 1], lhsT=esum_tot, rhs=ones_col,
                                 start=True, stop=True)
                rden = tinyp.tile([1, 1], F32, tag="rden")
                nc.vector.reciprocal(rden, ps_cd[:, D:D + 1])
                catt_tmp = tinyp.tile([1, D], F32, tag="catt_tmp")
                nc.vector.tensor_scalar_mul(catt_tmp, ps_cd[:, 0:D], rden)
                nc.sync.dma_start(out=catt_all[b:b + 1, :], in_=catt_tmp)

        if phase_limit < 3:
            nc.sync.dma_start(out=out_h[:, :], in_=h_loc)
            nc.sync.dma_start(out=out_c[:, :], in_=keyv_loc)
            nc.sync.dma_start(out=out_value[:, :], in_=scal_loc[:, 0:1])
            nc.sync.dma_start(out=out_logit[0:BL, 0:CONT], in_=xcont_loc)
            if phase_limit >= 2:
                nc.sync.dma_start(out=out_logit[0:BL, CONT:2 * CONT], in_=catt_all)

        if phase_limit >= 3:
            # ------------- phase 3: gate, LSTM, heads (local batch of 16) -------------
            g_scr = singles.tile([BL, CONT], F32, tag="g_scr")
            ga1 = singles.tile([BL, 1], F32, tag="ga1")
            ga2 = singles.tile([BL, 1], F32, tag="ga2")
            ga3 = singles.tile([BL, 1], F32, tag="ga3")
            gsum = singles.tile([BL, 1], F32, tag="gsum")
            nc.vector.scalar_tensor_tensor(out=g_scr, in0=catt_all, scalar=1.0,
                                           in1=watt_rep[:, 0:D], op0=ALU.mult,
                                           op1=ALU.mult, accum_out=ga1)
            nc.vector.scalar_tensor_tensor(out=g_scr, in0=h_loc, scalar=1.0,
                                           in1=watt_rep[:, D:D + LSTM], op0=ALU.mult,
                                           op1=ALU.mult, accum_out=ga2)
            nc.vector.scalar_tensor_tensor(out=g_scr, in0=xcont_loc, scalar=1.0,
                                           in1=watt_rep[:, D + LSTM:D + LSTM + CONT],
                                           op0=ALU.mult, op1=ALU.mult, accum_out=ga3)
            nc.vector.tensor_add(gsum, ga1, ga2)
            nc.vector.tensor_add(gsum, gsum, ga3)
            nc.vector.tensor_add(gsum, gsum, batt_rep)
            gate = singles.tile([BL, 1], F32, tag="gate")
            nc.scalar.activation(gate, gsum, AF.Sigmoid)
            att_sb = singles.tile([BL, D], F32, tag="att_sb")
            nc.vector.tensor_scalar_mul(att_sb, catt_all, gate)

            # LSTM (local)
            inpT_sb = singles.tile([128, 4, BL], BF16, tag="inpT_sb")
            hT_bf = singles.tile([128, 2, BL], BF16, tag="hT_bf")
            for lc in range(2):
                pe_transpose(inpT_sb[:, lc, :], att_sb[:, lc * 128:(lc + 1) * 128])
                pe_transpose(inpT_sb[:, 2 + lc, :], xcont_loc[:, lc * 128:(lc + 1) * 128])
                pe_transpose(hT_bf[:, lc, :], h_loc[:, lc * 128:(lc + 1) * 128])

            with tc.tile_pool(name="pg", bufs=1, space="PSUM") as pg, \
                 tc.tile_pool(name="pl", bufs=2, space="PSUM") as pl, \
                 tc.tile_pool(name="psm", bufs=1, space="PSUM") as psm:
                ps_g = pg.tile([BL, 4 * LSTM], F32, tag="ps_g")
                for nh in range(2):
                    sl = slice(nh * 512, (nh + 1) * 512)
                    for kc in range(4):
                        nc.tensor.matmul(ps_g[:, sl], lhsT=inpT_sb[:, kc, :],
                                         rhs=WihT_sb[:, kc, sl], start=(kc == 0), stop=False)
                    for lc in range(2):
                        nc.tensor.matmul(ps_g[:, sl], lhsT=hT_bf[:, lc, :],
                                         rhs=WhhT_sb[:, lc, sl], start=False, stop=(lc == 1))
                g_sb = singles.tile([BL, 4 * LSTM], F32, tag="g_sb")
                nc.vector.tensor_add(g_sb, ps_g, bih_rep)
                nc.vector.tensor_add(g_sb, g_sb, bhh_rep)

                i_s = singles.tile([BL, LSTM], F32, tag="i_s")
                f_s = singles.tile([BL, LSTM], F32, tag="f_s")
                gg_t = singles.tile([BL, LSTM], F32, tag="gg_t")
                o_s = singles.tile([BL, LSTM], F32, tag="o_s")
                nc.scalar.activation(i_s, g_sb[:, 0:256], AF.Sigmoid)
                nc.scalar.activation(f_s, g_sb[:, 256:512], AF.Sigmoid)
                nc.scalar.activation(gg_t, g_sb[:, 512:768], AF.Tanh)
                nc.scalar.activation(o_s, g_sb[:, 768:1024], AF.Sigmoid)

                m1 = singles.tile([BL, LSTM], F32, tag="m1")
                nc.vector.tensor_mul(m1, f_s, c_sb)
                m2 = singles.tile([BL, LSTM], F32, tag="m2")
                nc.vector.tensor_mul(m2, i_s, gg_t)
                c_out = singles.tile([BL, LSTM], F32, tag="c_out")
                nc.vector.tensor_add(c_out, m1, m2)
                tc_t = singles.tile([BL, LSTM], F32, tag="tc_t")
                nc.scalar.activation(tc_t, c_out, AF.Tanh)
                h_out = singles.tile([BL, LSTM], F32, tag="h_out")
                nc.vector.tensor_mul(h_out, o_s, tc_t)
                nc.sync.dma_start(out=out_h[:, :], in_=h_out)
                nc.sync.dma_start(out=out_c[:, :], in_=c_out)

                # actor hidden (local) then AllGather across cores
                houtT_sb = singles.tile([128, 2, BL], BF16, tag="houtT_sb")
                for lc in range(2):
                    pe_transpose(houtT_sb[:, lc, :], h_out[:, lc * 128:(lc + 1) * 128])
                ps_ah = psm.tile([BL, AH], F32, tag="ps_head")
                for lc in range(2):
                    nc.tensor.matmul(ps_ah, lhsT=houtT_sb[:, lc, :], rhs=WahT_sb[:, lc, :],
                                     start=(lc == 0), stop=(lc == 1))
                ha = singles.tile([BL, AH], F32, tag="ha")
                nc.vector.tensor_add(ha, ps_ah, bah_rep)
                nc.vector.tensor_scalar_max(ha, ha, 0.0)
                ag_in = dramp.tile([BL, AH], F32, tag="ag_in")
                nc.sync.dma_start(out=ag_in, in_=ha)
                ag_out = dramp.tile([B, AH], F32, tag="ag_out", addr_space="Shared")
                nc.gpsimd.collective_compute(
                    "AllGather", ALU.bypass, replica_groups=REPL,
                    ins=[ag_in[:, :]], outs=[ag_out[:, :]])
                ha_all = singles.tile([B, AH], F32, tag="ha_all")
                nc.sync.dma_start(out=ha_all, in_=ag_out)
                haT = singles.tile([128, B], BF16, tag="haT")
                pe_transpose(haT, ha_all)

                lsb = ctx.enter_context(tc.tile_pool(name="lsb", bufs=2))
                for so, slen in ASEGS:
                    sl = slice(so, so + slen)
                    ps_l = pl.tile([B, 512], F32, tag="ps_l")
                    nc.tensor.matmul(ps_l[:, :slen], lhsT=haT, rhs=WactT_sb[:, sl],
                                     start=True, stop=False)
                    nc.tensor.matmul(ps_l[:, :slen], lhsT=ones_row, rhs=bact_row[:, sl],
                                     start=False, stop=True)
                    l_sb = lsb.tile([B, 512], F32, tag="l_sb")
                    if so == 0:
                        nc.vector.tensor_copy(l_sb[:, :slen], ps_l[:, :slen])
                    else:
                        nc.scalar.copy(l_sb[:, :slen], ps_l[:, :slen])
                    nc.sync.dma_start(out=out_logit[:, sl], in_=l_sb[:, :slen])

                # critic head (local)
                ps_ch = psm.tile([BL, CH], F32, tag="ps_head")
                for lc in range(2):
                    nc.tensor.matmul(ps_ch, lhsT=houtT_sb[:, lc, :], rhs=WchT_sb[:, lc, :],
                                     start=(lc == 0), stop=(lc == 1))
                hc = singles.tile([BL, CH], F32, tag="hc")
                nc.vector.tensor_add(hc, ps_ch, bch_rep)
                nc.vector.tensor_scalar_max(hc, hc, 0.0)
                v_scr = singles.tile([BL, CH], F32, tag="v_scr")
                v_pre = singles.tile([BL, 1], F32, tag="v_pre")
                nc.vector.scalar_tensor_tensor(out=v_scr, in0=hc, scalar=1.0,
                                               in1=wcrit_rep, op0=ALU.mult, op1=ALU.mult,
                                               accum_out=v_pre)
                v_sb = singles.tile([BL, 1], F32, tag="v_sb")
                nc.vector.tensor_add(v_sb, v_pre, bcrit_rep)
                nc.sync.dma_start(out=out_value[:, :], in_=v_sb)

    if not nc.is_finalized():
        nc.finalize()
    return nc


_CACHE: dict = {}


def _get_nc() -> bass.Bass:
    if "nc" not in _CACHE:
        _CACHE["nc"] = build_nc()
    return _CACHE["nc"]


def make_in_maps(inputs: dict) -> list[dict]:
    ar = {k: np.ascontiguousarray(np.asarray(v, dtype=np.float32))
          for k, v in inputs.items()}
    in_maps = []
    for i in range(NCORES):
        sel = np.zeros((B, BL), np.float32)
        sel[i * BL:(i + 1) * BL, :] = np.eye(BL, dtype=np.float32)
        m = {
            "x_slice": np.ascontiguousarray(ar["x"][:, i * WSL:(i + 1) * WSL]),
            "feat": np.ascontiguousarray(ar["feat"][i * BL:(i + 1) * BL]),
            "h_tm1": ar["h_tm1"],
            "c_tm1": np.ascontiguousarray(ar["c_tm1"][i * BL:(i + 1) * BL]),
            "sel": sel,
            "W_x2cont_slice": np.ascontiguousarray(ar["W_x2cont"][:, i * WSL:(i + 1) * WSL]),
            "W_actor_slice": np.ascontiguousarray(ar["W_actor"][i * WSL:(i + 1) * WSL]),
            "b_actor_slice": np.ascontiguousarray(ar["b_actor"][i * WSL:(i + 1) * WSL]),
        }
        for k in ("b_x2cont", "W_h2cont", "b_h2cont", "W_cont2key", "b_cont2key",
                  "W_cont2strength", "b_cont2strength", "W_att_gates", "b_att_gates",
                  "W_ih", "b_ih", "W_hh", "b_hh", "W_actor_hid", "b_actor_hid",
                  "W_critic_hid", "b_critic_hid", "W_critic", "b_critic"):
            m[k] = ar[k]
        in_maps.append(m)
    return in_maps


def run(inputs: dict, **kwargs):
    nc = _get_nc()
    return run_bass_kernel_spmd(nc, make_in_maps(inputs), core_ids=list(range(NCORES)),
                                **kwargs)


def kernel(**inputs):
    res = run(inputs)
    r = res.results
    logits = np.concatenate([r[i]["action_logit"] for i in range(NCORES)], axis=1)
    value = np.concatenate([r[i]["value"] for i in range(NCORES)], axis=0)
    h = np.concatenate([r[i]["h"] for i in range(NCORES)], axis=0)
    c = np.concatenate([r[i]["c"] for i in range(NCORES)], axis=0)
    return (logits, value, h, c)


# revision 34
# speedup vs baseline: 1.1204x; 1.1204x over previous
"""Bass/Trainium2 kernel for nn_ACModel (NTM-style content addressing + LSTM + actor/critic).

Sharding (8 cores):
 - feat / c_tm1: data-parallel over batch (16 samples per core).
 - x / W_x2cont: k-split over the 10000-dim contraction (1250 per core) computed
   full-batch, combined with an AllReduce.
 - W_actor / b_actor: w-split over the 10000 output words (1250 per core), with an
   AllGather of the actor hidden layer; each core emits logits[:, its slice].
 - everything else replicated.
kernel(**inputs) takes FULL unsharded numpy inputs and returns (action_logit,
value, h, c) exactly like the reference.
"""

import numpy as np
from contextlib import ExitStack

import concourse.bass as bass
import concourse.bacc as bacc
import concourse.tile as tile
from concourse import mybir
from concourse.bass_utils import run_bass_kernel_spmd
from concourse.masks import make_identity

F32 = mybir.dt.float32
BF16 = mybir.dt.bfloat16
I32 = mybir.dt.int32
AF = mybir.ActivationFunctionType
ALU = mybir.AluOpType

# ---- problem shapes (hardcoded) ----
B, S, WN = 128, 4096, 10000
D = 256          # FEAT
CONT = 256
LSTM = 256
AH, CH = 128, 128
NCORES = 8
BL = B // NCORES          # 16 samples per core
NT = S // 128             # 32 slot-tiles of 128 per sample
HT = NT // 2
WSL = WN // NCORES        # 1250: per-core slice of the 10000 dim
# chunking of the per-core 1250-wide slice: 9 x 128 + 98
KCHUNKS = [(k * 128, 128) for k in range(9)] + [(1152, 98)]
# actor logit segments within the per-core 1250 columns (each <=512 psum bank)
ASEGS = [(0, 500), (500, 500), (1000, 250)]
# norm^2 tiles computed on DVE instead of ACT (load balancing)
DVE_NORM_TILES = (2, 7, 12, 17, 22, 27)
RSQRT_MAGIC = 0x5F3759DF
REPL = [list(range(NCORES))]


def _bcast_ap(src_ap: bass.AP, parts: int) -> bass.AP:
    return bass.AP(
        tensor=src_ap.tensor,
        offset=src_ap.offset,
        ap=[[0, parts]] + [list(p) for p in src_ap.ap],
    )


def build_nc(phase_limit: int = 3) -> bass.Bass:
    nc = bacc.Bacc()

    # ---------------- dram parameters ----------------
    xs_d = nc.declare_dram_parameter("x_slice", [B, WSL], F32, isOutput=False)
    feat_d = nc.declare_dram_parameter("feat", [BL, S, D], F32, isOutput=False)
    h_d = nc.declare_dram_parameter("h_tm1", [B, LSTM], F32, isOutput=False)
    c_d = nc.declare_dram_parameter("c_tm1", [BL, LSTM], F32, isOutput=False)
    sel_d = nc.declare_dram_parameter("sel", [B, BL], F32, isOutput=False)
    Wx2cs_d = nc.declare_dram_parameter("W_x2cont_slice", [CONT, WSL], F32, isOutput=False)
    bx2c_d = nc.declare_dram_parameter("b_x2cont", [CONT], F32, isOutput=False)
    Wh2c_d = nc.declare_dram_parameter("W_h2cont", [CONT, LSTM], F32, isOutput=False)
    bh2c_d = nc.declare_dram_parameter("b_h2cont", [CONT], F32, isOutput=False)
    Wc2k_d = nc.declare_dram_parameter("W_cont2key", [D, CONT], F32, isOutput=False)
    bc2k_d = nc.declare_dram_parameter("b_cont2key", [D], F32, isOutput=False)
    Wc2s_d = nc.declare_dram_parameter("W_cont2strength", [1, CONT], F32, isOutput=False)
    bc2s_d = nc.declare_dram_parameter("b_cont2strength", [1], F32, isOutput=False)
    Watt_d = nc.declare_dram_parameter("W_att_gates", [1, D + LSTM + CONT], F32, isOutput=False)
    batt_d = nc.declare_dram_parameter("b_att_gates", [1], F32, isOutput=False)
    Wih_d = nc.declare_dram_parameter("W_ih", [4 * LSTM, D + CONT], F32, isOutput=False)
    bih_d = nc.declare_dram_parameter("b_ih", [4 * LSTM], F32, isOutput=False)
    Whh_d = nc.declare_dram_parameter("W_hh", [4 * LSTM, LSTM], F32, isOutput=False)
    bhh_d = nc.declare_dram_parameter("b_hh", [4 * LSTM], F32, isOutput=False)
    Wah_d = nc.declare_dram_parameter("W_actor_hid", [AH, LSTM], F32, isOutput=False)
    bah_d = nc.declare_dram_parameter("b_actor_hid", [AH], F32, isOutput=False)
    Wch_d = nc.declare_dram_parameter("W_critic_hid", [CH, LSTM], F32, isOutput=False)
    bch_d = nc.declare_dram_parameter("b_critic_hid", [CH], F32, isOutput=False)
    Wacts_d = nc.declare_dram_parameter("W_actor_slice", [WSL, AH], F32, isOutput=False)
    bacts_d = nc.declare_dram_parameter("b_actor_slice", [WSL], F32, isOutput=False)
    Wcrit_d = nc.declare_dram_parameter("W_critic", [1, CH], F32, isOutput=False)
    bcrit_d = nc.declare_dram_parameter("b_critic", [1], F32, isOutput=False)

    out_logit = nc.declare_dram_parameter("action_logit", [B, WSL], F32, isOutput=True)
    out_value = nc.declare_dram_parameter("value", [BL, 1], F32, isOutput=True)
    out_h = nc.declare_dram_parameter("h", [BL, LSTM], F32, isOutput=True)
    out_c = nc.declare_dram_parameter("c", [BL, LSTM], F32, isOutput=True)

    with ExitStack() as ctx:
        tc = ctx.enter_context(tile.TileContext(nc))
        singles = ctx.enter_context(tc.tile_pool(name="singles", bufs=1))
        ptr = ctx.enter_context(tc.tile_pool(name="ptr", bufs=2, space="PSUM"))
        dramp = ctx.enter_context(tc.tile_pool(name="dramp", bufs=1, space="DRAM"))

        # ---------------- constants / staging ----------------
        identity = singles.tile([128, 128], F32, tag="identity")
        make_identity(nc, identity)
        ones_col = singles.tile([128, 1], F32, tag="ones_col")
        nc.vector.memset(ones_col, 1.0)
        ones_row = singles.tile([1, 128], F32, tag="ones_row")
        nc.vector.memset(ones_row, 1.0)

        h_sb = singles.tile([B, LSTM], F32, tag="h_sb")
        nc.sync.dma_start(out=h_sb, in_=h_d[:, :])
        c_sb = singles.tile([BL, LSTM], F32, tag="c_sb")
        nc.sync.dma_start(out=c_sb, in_=c_d[:, :])
        sel_sb = singles.tile([B, BL], F32, tag="sel_sb")
        nc.sync.dma_start(out=sel_sb, in_=sel_d[:, :])
        x_sb = singles.tile([B, WSL], F32, tag="x_sb")
        nc.sync.dma_start(out=x_sb, in_=xs_d[:, :])

        def bcast_tile(name, src_row_ap, parts, width):
            t = singles.tile([parts, width], F32, tag=name)
            nc.sync.dma_start(out=t, in_=_bcast_ap(src_row_ap, parts))
            return t

        bx2c_rep = bcast_tile("bx2c_rep", bx2c_d[:], B, CONT)
        bh2c_rep = bcast_tile("bh2c_rep", bh2c_d[:], B, CONT)
        bc2k_rep = bcast_tile("bc2k_rep", bc2k_d[:], B, D)
        wstr_rep = bcast_tile("wstr_rep", Wc2s_d[0, :], B, CONT)
        bc2s_rep = bcast_tile("bc2s_rep", bc2s_d[:], B, 1)
        watt_rep = bcast_tile("watt_rep", Watt_d[0, :], BL, D + LSTM + CONT)
        batt_rep = bcast_tile("batt_rep", batt_d[:], BL, 1)
        bih_rep = bcast_tile("bih_rep", bih_d[:], BL, 4 * LSTM)
        bhh_rep = bcast_tile("bhh_rep", bhh_d[:], BL, 4 * LSTM)
        bah_rep = bcast_tile("bah_rep", bah_d[:], BL, AH)
        bch_rep = bcast_tile("bch_rep", bch_d[:], BL, CH)
        wcrit_rep = bcast_tile("wcrit_rep", Wcrit_d[0, :], BL, CH)
        bcrit_rep = bcast_tile("bcrit_rep", bcrit_d[:], BL, 1)
        bact_row = singles.tile([1, WSL], F32, tag="bact_row")
        nc.sync.dma_start(out=bact_row, in_=bacts_d[:].rearrange("(a b) -> a b", a=1))

        # ---------------- helpers ----------------
        tr_count = [0]

        def pe_transpose(dst_ap, src_ap):
            p, f = src_ap.shape[0], src_ap.shape[1]
            pt = ptr.tile([128, 128], F32, tag="tr_ps")
            nc.tensor.transpose(pt[:f, :p], src_ap, identity[:p, :p])
            if tr_count[0] % 2 == 0:
                nc.vector.tensor_copy(dst_ap, pt[:f, :p])
            else:
                nc.scalar.copy(dst_ap, pt[:f, :p])
            tr_count[0] += 1

        def dve_rsqrt(dst, z, pool, tag, parts, width):
            """dst = 1/sqrt(z) elementwise via bit-hack + 2 Newton steps (DVE only)."""
            zb = z.bitcast(I32)
            sh = pool.tile([parts, width], I32, tag=tag + "_sh")
            nc.vector.tensor_scalar(out=sh, in0=zb, scalar1=1, scalar2=None,
                                    op0=ALU.logical_shift_right)
            nt = pool.tile([parts, width], I32, tag=tag + "_nt")
            nc.vector.tensor_scalar(out=nt, in0=sh, scalar1=0, scalar2=None,
                                    op0=ALU.bitwise_not)
            nc.vector.tensor_scalar(out=nt, in0=nt, scalar1=RSQRT_MAGIC + 1,
                                    scalar2=None, op0=ALU.add)
            y = nt.bitcast(F32)
            t = pool.tile([parts, width], F32, tag=tag + "_t")
            for it in range(2):
                nc.vector.tensor_mul(t, y, y)
                nc.vector.tensor_mul(t, t, z)
                nc.vector.tensor_scalar(out=t, in0=t, scalar1=-0.5, scalar2=1.5,
                                        op0=ALU.mult, op1=ALU.add)
                if it == 1:
                    nc.vector.tensor_mul(dst, y, t)
                else:
                    nc.vector.tensor_mul(y, y, t)

        # ---------------- one-time PE weight transposes ----------------
        # x_slice^T chunks: xT[:wsz, k, :] = x_slice[:, ko:ko+wsz]^T
        xT_sb = singles.tile([128, len(KCHUNKS), B], BF16, tag="xT_sb")
        for k, (ko, wsz) in enumerate(KCHUNKS):
            pe_transpose(xT_sb[:wsz, k, :], x_sb[:, ko:ko + wsz])

        # h^T (full batch, f32) for h_cont
        hT_sb = singles.tile([128, 2, B], F32, tag="hT_sb")
        for lc in range(2):
            pe_transpose(hT_sb[:, lc, :], h_sb[:, lc * 128:(lc + 1) * 128])

        # W_h2cont^T / W_cont2key^T  [128, lc, 256]
        Wh2c_st = singles.tile([128, 2, LSTM], F32, tag="Wh2c_st")
        Wc2k_st = singles.tile([128, 2, CONT], F32, tag="Wc2k_st")
        Wh2cT_sb = singles.tile([128, 2, CONT], F32, tag="Wh2cT_sb")
        Wc2kT_sb = singles.tile([128, 2, D], F32, tag="Wc2kT_sb")
        for mh in range(2):
            nc.sync.dma_start(out=Wh2c_st[:, mh, :], in_=Wh2c_d[mh * 128:(mh + 1) * 128, :])
            nc.sync.dma_start(out=Wc2k_st[:, mh, :], in_=Wc2k_d[mh * 128:(mh + 1) * 128, :])
        for mh in range(2):
            for lc in range(2):
                pe_transpose(Wh2cT_sb[:, lc, mh * 128:(mh + 1) * 128],
                             Wh2c_st[:, mh, lc * 128:(lc + 1) * 128])
                pe_transpose(Wc2kT_sb[:, lc, mh * 128:(mh + 1) * 128],
                             Wc2k_st[:, mh, lc * 128:(lc + 1) * 128])

        # W_x2cont_slice^T chunks
        wstage = ctx.enter_context(tc.tile_pool(name="wstage", bufs=4))
        wxp = ctx.enter_context(tc.tile_pool(name="wxp", bufs=2))
        Wx2cT_sb = singles.tile([128, len(KCHUNKS), CONT], BF16, tag="Wx2cT_sb")
        for mh in range(2):
            st = wxp.tile([128, WSL], F32, tag="wx_st")
            nc.sync.dma_start(out=st, in_=Wx2cs_d[mh * 128:(mh + 1) * 128, :])
            for k, (ko, wsz) in enumerate(KCHUNKS):
                pe_transpose(Wx2cT_sb[:wsz, k, mh * 128:(mh + 1) * 128],
                             st[:, ko:ko + wsz])

        # partial x_cont over this core's slice (launch the AllReduce early)
        pctrl = ctx.enter_context(tc.tile_pool(name="pctrl", bufs=2, space="PSUM"))
        ps_xc = pctrl.tile([B, CONT], F32, tag="ps_ctrl")
        for k, (ko, wsz) in enumerate(KCHUNKS):
            nc.tensor.matmul(ps_xc, lhsT=xT_sb[:wsz, k, :], rhs=Wx2cT_sb[:wsz, k, :],
                             start=(k == 0), stop=(k == len(KCHUNKS) - 1))
        xc_part = singles.tile([B, CONT], F32, tag="xc_part")
        nc.vector.tensor_copy(xc_part, ps_xc)
        cc_in = dramp.tile([B, CONT], F32, tag="cc_in")
        nc.sync.dma_start(out=cc_in, in_=xc_part)
        cc_out = dramp.tile([B, CONT], F32, tag="cc_out", addr_space="Shared")
        nc.gpsimd.collective_compute(
            "AllReduce", ALU.add, replica_groups=REPL,
            ins=[cc_in[:, :]], outs=[cc_out[:, :]])
        xc_full = singles.tile([B, CONT], F32, tag="xc_full")
        nc.sync.dma_start(out=xc_full, in_=cc_out)

        # ---------------- phase 1: controller (full batch of 128) ----------------
        xcont_sb = singles.tile([B, CONT], F32, tag="xcont_sb")
        ctrl_sb = singles.tile([B, CONT], F32, tag="ctrl_sb")
        keyv_sb = singles.tile([B, D], F32, tag="keyv_sb")
        keyv_loc = singles.tile([BL, D], F32, tag="keyv_loc")
        xcont_loc = singles.tile([BL, CONT], F32, tag="xcont_loc")
        h_loc = singles.tile([BL, LSTM], F32, tag="h_loc")
        cb_all = singles.tile([128, BL], F32, tag="cb_all")
        ns_all = singles.tile([128, BL], F32, tag="ns_all")
        scal_loc = singles.tile([BL, 2], F32, tag="scal_loc")

        if True:
            nc.vector.tensor_add(xcont_sb, xc_full, bx2c_rep)

            # h_cont + control
            ps_hc = pctrl.tile([B, CONT], F32, tag="ps_ctrl")
            for lc in range(2):
                nc.tensor.matmul(ps_hc, lhsT=hT_sb[:, lc, :], rhs=Wh2cT_sb[:, lc, :],
                                 start=(lc == 0), stop=(lc == 1))
            t1 = singles.tile([B, CONT], F32, tag="ph1_t1")
            nc.vector.tensor_add(t1, ps_hc, xcont_sb)
            nc.vector.tensor_add(t1, t1, bh2c_rep)
            nc.vector.tensor_scalar_max(ctrl_sb, t1, 0.0)

            ctrlT_sb = singles.tile([128, 2, B], F32, tag="ctrlT_sb")
            for lc in range(2):
                pe_transpose(ctrlT_sb[:, lc, :], ctrl_sb[:, lc * 128:(lc + 1) * 128])

            # key_v = tanh(control @ Wc2k^T + b)
            ps_kv = pctrl.tile([B, D], F32, tag="ps_ctrl")
            for lc in range(2):
                nc.tensor.matmul(ps_kv, lhsT=ctrlT_sb[:, lc, :], rhs=Wc2kT_sb[:, lc, :],
                                 start=(lc == 0), stop=(lc == 1))
            kv_pre = singles.tile([B, D], F32, tag="kv_pre")
            nc.vector.tensor_add(kv_pre, ps_kv, bc2k_rep)
            nc.scalar.activation(keyv_sb, kv_pre, AF.Tanh)

            # strength, c = strength * rsqrt(||key||^2), negstr
            str_scr = singles.tile([B, CONT], F32, tag="str_scr")
            str_pre = singles.tile([B, 1], F32, tag="str_pre")
            nc.vector.scalar_tensor_tensor(out=str_scr, in0=ctrl_sb, scalar=1.0,
                                           in1=wstr_rep, op0=ALU.mult, op1=ALU.mult,
                                           accum_out=str_pre)
            strength = singles.tile([B, 1], F32, tag="strength")
            nc.vector.tensor_add(strength, str_pre, bc2s_rep)
            nc.vector.tensor_scalar_max(strength, strength, 0.0)
            nc.vector.tensor_scalar_add(strength, strength, 1.0)
            kn_scr = singles.tile([B, D], F32, tag="kn_scr")
            kn2 = singles.tile([B, 1], F32, tag="kn2")
            nc.scalar.activation(kn_scr, keyv_sb, AF.Square, accum_out=kn2)
            kn_rs = singles.tile([B, 1], F32, tag="kn_rs")
            dve_rsqrt(kn_rs, kn2, singles, "knrs", B, 1)
            pack = singles.tile([B, 2], F32, tag="pack")
            nc.vector.tensor_mul(pack[:, 0:1], strength, kn_rs)
            nc.vector.tensor_scalar_mul(pack[:, 1:2], strength, -1.0)

            # extract local rows: loc = sel^T @ full
            def extract(dst, src, width):
                ps = pctrl.tile([BL, CONT], F32, tag="ps_ctrl")
                nc.tensor.matmul(ps[:, :width], lhsT=sel_sb, rhs=src,
                                 start=True, stop=True)
                nc.vector.tensor_copy(dst, ps[:, :width])

            extract(keyv_loc, keyv_sb, D)
            extract(xcont_loc, xcont_sb, CONT)
            extract(h_loc, h_sb, LSTM)
            extract(scal_loc, pack, 2)

            # per-sample exp scale/bias broadcast across partitions
            for col, dst, tg in ((0, cb_all, "cbr"), (1, ns_all, "nsr")):
                ptc = ptr.tile([128, 128], F32, tag="tr_ps")
                nc.tensor.transpose(ptc[:1, :BL], scal_loc[:, col:col + 1],
                                    identity[:BL, :BL])
                row = singles.tile([1, BL], F32, tag=tg)
                nc.vector.tensor_copy(row, ptc[:1, :BL])
                nc.gpsimd.partition_broadcast(dst, row)


        # W_ih^T / W_hh^T (bf16, phase 3)
        WihT_sb = singles.tile([128, 4, 4 * LSTM], BF16, tag="WihT_sb")
        for gc in range(8):
            st = wstage.tile([128, D + CONT], F32, tag="wih_st")
            nc.sync.dma_start(out=st, in_=Wih_d[gc * 128:(gc + 1) * 128, :])
            for kc in range(4):
                pe_transpose(WihT_sb[:, kc, gc * 128:(gc + 1) * 128],
                             st[:, kc * 128:(kc + 1) * 128])
        WhhT_sb = singles.tile([128, 2, 4 * LSTM], BF16, tag="WhhT_sb")
        for gc in range(8):
            st = wstage.tile([128, LSTM], F32, tag="whh_st")
            nc.sync.dma_start(out=st, in_=Whh_d[gc * 128:(gc + 1) * 128, :])
            for lc in range(2):
                pe_transpose(WhhT_sb[:, lc, gc * 128:(gc + 1) * 128],
                             st[:, lc * 128:(lc + 1) * 128])
        WahT_sb = singles.tile([128, 2, AH], BF16, tag="WahT_sb")
        WchT_sb = singles.tile([128, 2, CH], BF16, tag="WchT_sb")
        st_ah = singles.tile([AH, LSTM], F32, tag="st_ah")
        nc.sync.dma_start(out=st_ah, in_=Wah_d[:, :])
        st_ch = singles.tile([CH, LSTM], F32, tag="st_ch")
        nc.sync.dma_start(out=st_ch, in_=Wch_d[:, :])
        for lc in range(2):
            pe_transpose(WahT_sb[:, lc, :], st_ah[:, lc * 128:(lc + 1) * 128])
            pe_transpose(WchT_sb[:, lc, :], st_ch[:, lc * 128:(lc + 1) * 128])
        # W_actor_slice^T (bf16) [128 ah, 1250]
        WactT_sb = singles.tile([128, WSL], BF16, tag="WactT_sb")
        for k, (ko, wsz) in enumerate(KCHUNKS):
            st = wstage.tile([128, AH], F32, tag="wact_st")
            nc.sync.dma_start(out=st[:wsz, :], in_=Wacts_d[ko:ko + wsz, :])
            pe_transpose(WactT_sb[:, ko:ko + wsz], st[:wsz, :])


        # ---------------- phase 2: memory read (per sample streaming) ----------------
        catt_all = singles.tile([BL, D], F32, tag="catt_all")
        keyv_dram = dramp.tile([BL, D], F32, tag="keyv_dram")
        nc.sync.dma_start(out=keyv_dram, in_=keyv_loc)

        featp = ctx.enter_context(tc.tile_pool(name="featp", bufs=4))
        keyrp = ctx.enter_context(tc.tile_pool(name="keyrp", bufs=3))
        scrp = ctx.enter_context(tc.tile_pool(name="scrp", bufs=2))
        smallp = ctx.enter_context(tc.tile_pool(name="smallp", bufs=4))
        tinyp = ctx.enter_context(tc.tile_pool(name="tinyp", bufs=2))
        with tc.tile_pool(name="pcand", bufs=2, space="PSUM") as pcand:
            for b in (range(BL) if phase_limit >= 2 else []):
                key_rep = keyrp.tile([128, D], F32, tag="key_rep")
                nc.sync.dma_start(out=key_rep, in_=_bcast_ap(keyv_dram[b], 128))
                cb = cb_all[:, b:b + 1]
                nsb = ns_all[:, b:b + 1]

                fsrc = feat_d[b].rearrange("(p j) d -> p j d", p=128)
                fhalves = []
                for hf in range(2):
                    fh = featp.tile([128, HT, D], F32, tag="fbh")
                    for g in range(2):
                        go = hf * HT + g * 8
                        nc.sync.dma_start(out=fh[:, g * 8:(g + 1) * 8, :],
                                          in_=fsrc[:, go:go + 8, :])
                    fhalves.append(fh)

                def fbt(t):
                    return fhalves[t // HT][:, t % HT, :]

                ps_cd = pcand.tile([1, D + 1], F32, tag="ps_cd")
                for hf in range(2):
                    dots = smallp.tile([128, HT], F32, tag="dots")
                    nrm2 = smallp.tile([128, HT], F32, tag="nrm2")
                    for tl in range(HT):
                        t = hf * HT + tl
                        scr = scrp.tile([128, D], F32, tag="scr_dve")
                        nc.vector.scalar_tensor_tensor(
                            out=scr, in0=fbt(t), scalar=1.0, in1=key_rep,
                            op0=ALU.mult, op1=ALU.mult, accum_out=dots[:, tl:tl + 1])
                        if t in DVE_NORM_TILES:
                            scr2 = scrp.tile([128, D], F32, tag="scr_dve2")
                            nc.vector.scalar_tensor_tensor(
                                out=scr2, in0=fbt(t), scalar=1.0, in1=fbt(t),
                                op0=ALU.mult, op1=ALU.mult,
                                accum_out=nrm2[:, tl:tl + 1])
                        else:
                            scr3 = scrp.tile([128, D], F32, tag="sq_scr")
                            nc.scalar.activation(scr3, fbt(t), AF.Square,
                                                 accum_out=nrm2[:, tl:tl + 1])

                    rs = smallp.tile([128, HT], F32, tag="rs")
                    dve_rsqrt(rs, nrm2, smallp, "p2rs", 128, HT)
                    u = smallp.tile([128, HT], F32, tag="u")
                    nc.vector.tensor_mul(u, dots, rs)
                    e = smallp.tile([128, HT], F32, tag="e")
                    if hf == 0:
                        esum_all = smallp.tile([128, 2], F32, tag="esum_all")
                    nc.scalar.activation(e, u, AF.Exp, scale=cb, bias=nsb,
                                         accum_out=esum_all[:, hf:hf + 1])

                    for tl in range(HT):
                        t = hf * HT + tl
                        nc.tensor.matmul(ps_cd[:, 0:D], lhsT=e[:, tl:tl + 1],
                                         rhs=fbt(t), start=(t == 0), stop=(t == NT - 1))
                esum_tot = smallp.tile([128, 1], F32, tag="esum_tot")
                nc.vector.tensor_reduce(out=esum_tot, in_=esum_all,
                                        axis=mybir.AxisListType.X, op=ALU.add)
                nc.tensor.matmul(ps_cd[:, D:D + 1], lhsT=esum_tot, rhs=ones_col,
                                 start=True, stop=True)
                rden = tinyp.tile([1, 1], F32, tag="rden")
                nc.vector.reciprocal(rden, ps_cd[:, D:D + 1])
                catt_tmp = tinyp.tile([1, D], F32, tag="catt_tmp")
                nc.vector.tensor_scalar_mul(catt_tmp, ps_cd[:, 0:D], rden)
                nc.sync.dma_start(out=catt_all[b:b + 1, :], in_=catt_tmp)

        if phase_limit < 3:
            nc.sync.dma_start(out=out_h[:, :], in_=h_loc)
            nc.sync.dma_start(out=out_c[:, :], in_=keyv_loc)
            nc.sync.dma_start(out=out_value[:, :], in_=scal_loc[:, 0:1])
            nc.sync.dma_start(out=out_logit[0:BL, 0:CONT], in_=xcont_loc)
            if phase_limit >= 2:
                nc.sync.dma_start(out=out_logit[0:BL, CONT:2 * CONT], in_=catt_all)

        if phase_limit >= 3:
            # ------------- phase 3: gate, LSTM, heads (local batch of 16) -------------
            g_scr = singles.tile([BL, CONT], F32, tag="g_scr")
            ga1 = singles.tile([BL, 1], F32, tag="ga1")
            ga2 = singles.tile([BL, 1], F32, tag="ga2")
            ga3 = singles.tile([BL, 1], F32, tag="ga3")
            gsum = singles.tile([BL, 1], F32, tag="gsum")
            nc.vector.scalar_tensor_tensor(out=g_scr, in0=catt_all, scalar=1.0,
                                           in1=watt_rep[:, 0:D], op0=ALU.mult,
                                           op1=ALU.mult, accum_out=ga1)
            nc.vector.scalar_tensor_tensor(out=g_scr, in0=h_loc, scalar=1.0,
                                           in1=watt_rep[:, D:D + LSTM], op0=ALU.mult,
                                           op1=ALU.mult, accum_out=ga2)
            nc.vector.scalar_tensor_tensor(out=g_scr, in0=xcont_loc, scalar=1.0,
                                           in1=watt_rep[:, D + LSTM:D + LSTM + CONT],
                                           op0=ALU.mult, op1=ALU.mult, accum_out=ga3)
            nc.vector.tensor_add(gsum, ga1, ga2)
            nc.vector.tensor_add(gsum, gsum, ga3)
            nc.vector.tensor_add(gsum, gsum, batt_rep)
            gate = singles.tile([BL, 1], F32, tag="gate")
            nc.scalar.activation(gate, gsum, AF.Sigmoid)
            att_sb = singles.tile([BL, D], F32, tag="att_sb")
            nc.vector.tensor_scalar_mul(att_sb, catt_all, gate)

            # LSTM (local)
            inpT_sb = singles.tile([128, 4, BL], BF16, tag="inpT_sb")
            hT_bf = singles.tile([128, 2, BL], BF16, tag="hT_bf")
            for lc in range(2):
                pe_transpose(inpT_sb[:, lc, :], att_sb[:, lc * 128:(lc + 1) * 128])
                pe_transpose(inpT_sb[:, 2 + lc, :], xcont_loc[:, lc * 128:(lc + 1) * 128])
                pe_transpose(hT_bf[:, lc, :], h_loc[:, lc * 128:(lc + 1) * 128])

            with tc.tile_pool(name="pg", bufs=1, space="PSUM") as pg, \
                 tc.tile_pool(name="pl", bufs=1, space="PSUM") as pl, \
                 tc.tile_pool(name="psm", bufs=1, space="PSUM") as psm:
                ps_g = pg.tile([BL, 4 * LSTM], F32, tag="ps_g")
                for nh in range(2):
                    sl = slice(nh * 512, (nh + 1) * 512)
                    for kc in range(4):
                        nc.tensor.matmul(ps_g[:, sl], lhsT=inpT_sb[:, kc, :],
                                         rhs=WihT_sb[:, kc, sl], start=(kc == 0), stop=False)
                    for lc in range(2):
                        nc.tensor.matmul(ps_g[:, sl], lhsT=hT_bf[:, lc, :],
                                         rhs=WhhT_sb[:, lc, sl], start=False, stop=(lc == 1))
                g_sb = singles.tile([BL, 4 * LSTM], F32, tag="g_sb")
                nc.vector.tensor_add(g_sb, ps_g, bih_rep)
                nc.vector.tensor_add(g_sb, g_sb, bhh_rep)

                i_s = singles.tile([BL, LSTM], F32, tag="i_s")
                f_s = singles.tile([BL, LSTM], F32, tag="f_s")
                gg_t = singles.tile([BL, LSTM], F32, tag="gg_t")
                o_s = singles.tile([BL, LSTM], F32, tag="o_s")
                nc.scalar.activation(i_s, g_sb[:, 0:256], AF.Sigmoid)
                nc.scalar.activation(f_s, g_sb[:, 256:512], AF.Sigmoid)
                nc.scalar.activation(gg_t, g_sb[:, 512:768], AF.Tanh)
                nc.scalar.activation(o_s, g_sb[:, 768:1024], AF.Sigmoid)

                m1 = singles.tile([BL, LSTM], F32, tag="m1")
                nc.vector.tensor_mul(m1, f_s, c_sb)
                m2 = singles.tile([BL, LSTM], F32, tag="m2")
                nc.vector.tensor_mul(m2, i_s, gg_t)
                c_out = singles.tile([BL, LSTM], F32, tag="c_out")
                nc.vector.tensor_add(c_out, m1, m2)
                tc_t = singles.tile([BL, LSTM], F32, tag="tc_t")
                nc.scalar.activation(tc_t, c_out, AF.Tanh)
                h_out = singles.tile([BL, LSTM], F32, tag="h_out")
                nc.vector.tensor_mul(h_out, o_s, tc_t)
                nc.sync.dma_start(out=out_h[:, :], in_=h_out)
                nc.sync.dma_start(out=out_c[:, :], in_=c_out)

                # actor hidden (local) then AllGather across cores
                houtT_sb = singles.tile([128, 2, BL], BF16, tag="houtT_sb")
                for lc in range(2):
                    pe_transpose(houtT_sb[:, lc, :], h_out[:, lc * 128:(lc + 1) * 128])
                ps_ah = psm.tile([BL, AH], F32, tag="ps_head")
                for lc in range(2):
                    nc.tensor.matmul(ps_ah, lhsT=houtT_sb[:, lc, :], rhs=WahT_sb[:, lc, :],
                                     start=(lc == 0), stop=(lc == 1))
                ha = singles.tile([BL, AH], F32, tag="ha")
                nc.vector.tensor_add(ha, ps_ah, bah_rep)
                nc.vector.tensor_scalar_max(ha, ha, 0.0)
                ag_in = dramp.tile([BL, AH], F32, tag="ag_in")
                nc.sync.dma_start(out=ag_in, in_=ha)
                ag_out = dramp.tile([B, AH], F32, tag="ag_out", addr_space="Shared")
                nc.gpsimd.collective_compute(
                    "AllGather", ALU.bypass, replica_groups=REPL,
                    ins=[ag_in[:, :]], outs=[ag_out[:, :]])
                ha_all = singles.tile([B, AH], F32, tag="ha_all")
                nc.sync.dma_start(out=ha_all, in_=ag_out)
                haT = singles.tile([128, B], BF16, tag="haT")
                pe_transpose(haT, ha_all)

                lsb = ctx.enter_context(tc.tile_pool(name="lsb", bufs=2))
                for so, slen in ASEGS:
                    sl = slice(so, so + slen)
                    ps_l = pl.tile([B, 512], F32, tag="ps_l")
                    nc.tensor.matmul(ps_l[:, :slen], lhsT=haT, rhs=WactT_sb[:, sl],
                                     start=True, stop=False)
                    nc.tensor.matmul(ps_l[:, :slen], lhsT=ones_row, rhs=bact_row[:, sl],
                                     start=False, stop=True)
                    l_sb = lsb.tile([B, 512], F32, tag="l_sb")
                    if so == 0:
                        nc.vector.tensor_copy(l_sb[:, :slen], ps_l[:, :slen])
                    else:
                        nc.scalar.copy(l_sb[:, :slen], ps_l[:, :slen])
                    nc.sync.dma_start(out=out_logit[:, sl], in_=l_sb[:, :slen])

                # critic head (local)
                ps_ch = psm.tile([BL, CH], F32, tag="ps_head")
                for lc in range(2):
                    nc.tensor.matmul(ps_ch, lhsT=houtT_sb[:, lc, :], rhs=WchT_sb[:, lc, :],
                                     start=(lc == 0), stop=(lc == 1))
                hc = singles.tile([BL, CH], F32, tag="hc")
                nc.vector.tensor_add(hc, ps_ch, bch_rep)
                nc.vector.tensor_scalar_max(hc, hc, 0.0)
                v_scr = singles.tile([BL, CH], F32, tag="v_scr")
                v_pre = singles.tile([BL, 1], F32, tag="v_pre")
                nc.vector.scalar_tensor_tensor(out=v_scr, in0=hc, scalar=1.0,
                                               in1=wcrit_rep, op0=ALU.mult, op1=ALU.mult,
                                               accum_out=v_pre)
                v_sb = singles.tile([BL, 1], F32, tag="v_sb")
                nc.vector.tensor_add(v_sb, v_pre, bcrit_rep)
                nc.sync.dma_start(out=out_value[:, :], in_=v_sb)

    if not nc.is_finalized():
        nc.finalize()
    return nc


_CACHE: dict = {}


def _get_nc() -> bass.Bass:
    if "nc" not in _CACHE:
        _CACHE["nc"] = build_nc()
    return _CACHE["nc"]


def make_in_maps(inputs: dict) -> list[dict]:
    ar = {k: np.ascontiguousarray(np.asarray(v, dtype=np.float32))
          for k, v in inputs.items()}
    in_maps = []
    for i in range(NCORES):
        sel = np.zeros((B, BL), np.float32)
        sel[i * BL:(i + 1) * BL, :] = np.eye(BL, dtype=np.float32)
        m = {
            "x_slice": np.ascontiguousarray(ar["x"][:, i * WSL:(i + 1) * WSL]),
            "feat": np.ascontiguousarray(ar["feat"][i * BL:(i + 1) * BL]),
            "h_tm1": ar["h_tm1"],
            "c_tm1": np.ascontiguousarray(ar["c_tm1"][i * BL:(i + 1) * BL]),
            "sel": sel,
            "W_x2cont_slice": np.ascontiguousarray(ar["W_x2cont"][:, i * WSL:(i + 1) * WSL]),
            "W_actor_slice": np.ascontiguousarray(ar["W_actor"][i * WSL:(i + 1) * WSL]),
            "b_actor_slice": np.ascontiguousarray(ar["b_actor"][i * WSL:(i + 1) * WSL]),
        }
        for k in ("b_x2cont", "W_h2cont", "b_h2cont", "W_cont2key", "b_cont2key",
                  "W_cont2strength", "b_cont2strength", "W_att_gates", "b_att_gates",
                  "W_ih", "b_ih", "W_hh", "b_hh", "W_actor_hid", "b_actor_hid",
                  "W_critic_hid", "b_critic_hid", "W_critic", "b_critic"):
            m[k] = ar[k]
        in_maps.append(m)
    return in_maps


def run(inputs: dict, **kwargs):
    nc = _get_nc()
    return run_bass_kernel_spmd(nc, make_in_maps(inputs), core_ids=list(range(NCORES)),
                                **kwargs)


def kernel(**inputs):
    res = run(inputs)
    r = res.results
    logits = np.concatenate([r[i]["action_logit"] for i in range(NCORES)], axis=1)
    value = np.concatenate([r[i]["value"] for i in range(NCORES)], axis=0)
    h = np.concatenate([r[i]["h"] for i in range(NCORES)], axis=0)
    c = np.concatenate([r[i]["c"] for i in range(NCORES)], axis=0)
    return (logits, value, h, c)


# revision 36
# speedup vs baseline: 1.1222x; 1.0016x over previous
"""Bass/Trainium2 kernel for nn_ACModel (NTM-style content addressing + LSTM + actor/critic).

Sharding (8 cores):
 - feat / c_tm1: data-parallel over batch (16 samples per core).
 - x / W_x2cont: k-split over the 10000-dim contraction (1250 per core) computed
   full-batch, combined with an AllReduce.
 - W_actor / b_actor: w-split over the 10000 output words (1250 per core), with an
   AllGather of the actor hidden layer; each core emits logits[:, its slice].
 - everything else replicated.
kernel(**inputs) takes FULL unsharded numpy inputs and returns (action_logit,
value, h, c) exactly like the reference.
"""

import numpy as np
from contextlib import ExitStack

import concourse.bass as bass
import concourse.bacc as bacc
import concourse.tile as tile
from concourse import mybir
from concourse.bass_utils import run_bass_kernel_spmd
from concourse.masks import make_identity

F32 = mybir.dt.float32
BF16 = mybir.dt.bfloat16
I32 = mybir.dt.int32
AF = mybir.ActivationFunctionType
ALU = mybir.AluOpType

# ---- problem shapes (hardcoded) ----
B, S, WN = 128, 4096, 10000
D = 256          # FEAT
CONT = 256
LSTM = 256
AH, CH = 128, 128
NCORES = 8
BL = B // NCORES          # 16 samples per core
NT = S // 128             # 32 slot-tiles of 128 per sample
HT = NT // 2
WSL = WN // NCORES        # 1250: per-core slice of the 10000 dim
# chunking of the per-core 1250-wide slice: 9 x 128 + 98
KCHUNKS = [(k * 128, 128) for k in range(9)] + [(1152, 98)]
# actor logit segments within the per-core 1250 columns (each <=512 psum bank)
ASEGS = [(0, 500), (500, 500), (1000, 250)]
# norm^2 tiles computed on DVE instead of ACT (load balancing)
DVE_NORM_TILES = (2, 7, 12, 17, 22, 27)
RSQRT_MAGIC = 0x5F3759DF
REPL = [list(range(NCORES))]


def _bcast_ap(src_ap: bass.AP, parts: int) -> bass.AP:
    return bass.AP(
        tensor=src_ap.tensor,
        offset=src_ap.offset,
        ap=[[0, parts]] + [list(p) for p in src_ap.ap],
    )


def build_nc(phase_limit: int = 3) -> bass.Bass:
    nc = bacc.Bacc()

    # ---------------- dram parameters ----------------
    xs_d = nc.declare_dram_parameter("x_slice", [B, WSL], F32, isOutput=False)
    feat_d = nc.declare_dram_parameter("feat", [BL, S, D], F32, isOutput=False)
    h_d = nc.declare_dram_parameter("h_tm1", [BL, LSTM], F32, isOutput=False)
    c_d = nc.declare_dram_parameter("c_tm1", [BL, LSTM], F32, isOutput=False)
    Wx2cs_d = nc.declare_dram_parameter("W_x2cont_slice", [CONT, WSL], F32, isOutput=False)
    bx2c_d = nc.declare_dram_parameter("b_x2cont", [CONT], F32, isOutput=False)
    Wh2c_d = nc.declare_dram_parameter("W_h2cont", [CONT, LSTM], F32, isOutput=False)
    bh2c_d = nc.declare_dram_parameter("b_h2cont", [CONT], F32, isOutput=False)
    Wc2k_d = nc.declare_dram_parameter("W_cont2key", [D, CONT], F32, isOutput=False)
    bc2k_d = nc.declare_dram_parameter("b_cont2key", [D], F32, isOutput=False)
    Wc2s_d = nc.declare_dram_parameter("W_cont2strength", [1, CONT], F32, isOutput=False)
    bc2s_d = nc.declare_dram_parameter("b_cont2strength", [1], F32, isOutput=False)
    Watt_d = nc.declare_dram_parameter("W_att_gates", [1, D + LSTM + CONT], F32, isOutput=False)
    batt_d = nc.declare_dram_parameter("b_att_gates", [1], F32, isOutput=False)
    Wih_d = nc.declare_dram_parameter("W_ih", [4 * LSTM, D + CONT], F32, isOutput=False)
    bih_d = nc.declare_dram_parameter("b_ih", [4 * LSTM], F32, isOutput=False)
    Whh_d = nc.declare_dram_parameter("W_hh", [4 * LSTM, LSTM], F32, isOutput=False)
    bhh_d = nc.declare_dram_parameter("b_hh", [4 * LSTM], F32, isOutput=False)
    Wah_d = nc.declare_dram_parameter("W_actor_hid", [AH, LSTM], F32, isOutput=False)
    bah_d = nc.declare_dram_parameter("b_actor_hid", [AH], F32, isOutput=False)
    Wch_d = nc.declare_dram_parameter("W_critic_hid", [CH, LSTM], F32, isOutput=False)
    bch_d = nc.declare_dram_parameter("b_critic_hid", [CH], F32, isOutput=False)
    Wacts_d = nc.declare_dram_parameter("W_actor_slice", [WSL, AH], F32, isOutput=False)
    bacts_d = nc.declare_dram_parameter("b_actor_slice", [WSL], F32, isOutput=False)
    Wcrit_d = nc.declare_dram_parameter("W_critic", [1, CH], F32, isOutput=False)
    bcrit_d = nc.declare_dram_parameter("b_critic", [1], F32, isOutput=False)

    out_logit = nc.declare_dram_parameter("action_logit", [B, WSL], F32, isOutput=True)
    out_value = nc.declare_dram_parameter("value", [BL, 1], F32, isOutput=True)
    out_h = nc.declare_dram_parameter("h", [BL, LSTM], F32, isOutput=True)
    out_c = nc.declare_dram_parameter("c", [BL, LSTM], F32, isOutput=True)

    with ExitStack() as ctx:
        tc = ctx.enter_context(tile.TileContext(nc))
        singles = ctx.enter_context(tc.tile_pool(name="singles", bufs=1))
        ptr = ctx.enter_context(tc.tile_pool(name="ptr", bufs=2, space="PSUM"))
        dramp = ctx.enter_context(tc.tile_pool(name="dramp", bufs=1, space="DRAM"))

        # ---------------- constants / staging ----------------
        identity = singles.tile([128, 128], F32, tag="identity")
        make_identity(nc, identity)
        ones_col = singles.tile([128, 1], F32, tag="ones_col")
        nc.vector.memset(ones_col, 1.0)
        ones_row = singles.tile([1, 128], F32, tag="ones_row")
        nc.vector.memset(ones_row, 1.0)

        h_sb = singles.tile([BL, LSTM], F32, tag="h_sb")
        nc.sync.dma_start(out=h_sb, in_=h_d[:, :])
        c_sb = singles.tile([BL, LSTM], F32, tag="c_sb")
        nc.sync.dma_start(out=c_sb, in_=c_d[:, :])
        x_sb = singles.tile([B, WSL], F32, tag="x_sb")
        nc.sync.dma_start(out=x_sb, in_=xs_d[:, :])

        def bcast_tile(name, src_row_ap, parts, width):
            t = singles.tile([parts, width], F32, tag=name)
            nc.sync.dma_start(out=t, in_=_bcast_ap(src_row_ap, parts))
            return t

        bx2c_rep = bcast_tile("bx2c_rep", bx2c_d[:], BL, CONT)
        bh2c_rep = bcast_tile("bh2c_rep", bh2c_d[:], BL, CONT)
        bc2k_rep = bcast_tile("bc2k_rep", bc2k_d[:], BL, D)
        wstr_rep = bcast_tile("wstr_rep", Wc2s_d[0, :], BL, CONT)
        bc2s_rep = bcast_tile("bc2s_rep", bc2s_d[:], BL, 1)
        watt_rep = bcast_tile("watt_rep", Watt_d[0, :], BL, D + LSTM + CONT)
        batt_rep = bcast_tile("batt_rep", batt_d[:], BL, 1)
        bih_rep = bcast_tile("bih_rep", bih_d[:], BL, 4 * LSTM)
        bhh_rep = bcast_tile("bhh_rep", bhh_d[:], BL, 4 * LSTM)
        bah_rep = bcast_tile("bah_rep", bah_d[:], BL, AH)
        bch_rep = bcast_tile("bch_rep", bch_d[:], BL, CH)
        wcrit_rep = bcast_tile("wcrit_rep", Wcrit_d[0, :], BL, CH)
        bcrit_rep = bcast_tile("bcrit_rep", bcrit_d[:], BL, 1)
        bact_row = singles.tile([1, WSL], F32, tag="bact_row")
        nc.sync.dma_start(out=bact_row, in_=bacts_d[:].rearrange("(a b) -> a b", a=1))

        # ---------------- helpers ----------------
        tr_count = [0]

        def pe_transpose(dst_ap, src_ap):
            p, f = src_ap.shape[0], src_ap.shape[1]
            pt = ptr.tile([128, 128], F32, tag="tr_ps")
            nc.tensor.transpose(pt[:f, :p], src_ap, identity[:p, :p])
            if tr_count[0] % 2 == 0:
                nc.vector.tensor_copy(dst_ap, pt[:f, :p])
            else:
                nc.scalar.copy(dst_ap, pt[:f, :p])
            tr_count[0] += 1

        def dve_rsqrt(dst, z, pool, tag, parts, width, iters=2):
            """dst = 1/sqrt(z) elementwise via bit-hack + Newton steps (DVE only)."""
            zb = z.bitcast(I32)
            sh = pool.tile([parts, width], I32, tag=tag + "_sh")
            nc.vector.tensor_scalar(out=sh, in0=zb, scalar1=1, scalar2=None,
                                    op0=ALU.logical_shift_right)
            nt = pool.tile([parts, width], I32, tag=tag + "_nt")
            nc.vector.tensor_scalar(out=nt, in0=sh, scalar1=0, scalar2=None,
                                    op0=ALU.bitwise_not)
            nc.vector.tensor_scalar(out=nt, in0=nt, scalar1=RSQRT_MAGIC + 1,
                                    scalar2=None, op0=ALU.add)
            y = nt.bitcast(F32)
            t = pool.tile([parts, width], F32, tag=tag + "_t")
            for it in range(iters):
                nc.vector.tensor_mul(t, y, y)
                nc.vector.tensor_mul(t, t, z)
                nc.vector.tensor_scalar(out=t, in0=t, scalar1=-0.5, scalar2=1.5,
                                        op0=ALU.mult, op1=ALU.add)
                if it == iters - 1:
                    nc.vector.tensor_mul(dst, y, t)
                else:
                    nc.vector.tensor_mul(y, y, t)

        # ---------------- one-time PE weight transposes ----------------
        # x_slice^T chunks: xT[:wsz, k, :] = x_slice[:, ko:ko+wsz]^T
        xT_sb = singles.tile([128, len(KCHUNKS), B], BF16, tag="xT_sb")
        for k, (ko, wsz) in enumerate(KCHUNKS):
            pe_transpose(xT_sb[:wsz, k, :], x_sb[:, ko:ko + wsz])

        # h^T (local, f32) for h_cont
        hT_sb = singles.tile([128, 2, BL], F32, tag="hT_sb")
        for lc in range(2):
            pe_transpose(hT_sb[:, lc, :], h_sb[:, lc * 128:(lc + 1) * 128])

        # W_h2cont^T / W_cont2key^T  [128, lc, 256]
        Wh2c_st = singles.tile([128, 2, LSTM], F32, tag="Wh2c_st")
        Wc2k_st = singles.tile([128, 2, CONT], F32, tag="Wc2k_st")
        Wh2cT_sb = singles.tile([128, 2, CONT], F32, tag="Wh2cT_sb")
        Wc2kT_sb = singles.tile([128, 2, D], F32, tag="Wc2kT_sb")
        for mh in range(2):
            nc.sync.dma_start(out=Wh2c_st[:, mh, :], in_=Wh2c_d[mh * 128:(mh + 1) * 128, :])
            nc.sync.dma_start(out=Wc2k_st[:, mh, :], in_=Wc2k_d[mh * 128:(mh + 1) * 128, :])
        for mh in range(2):
            for lc in range(2):
                pe_transpose(Wh2cT_sb[:, lc, mh * 128:(mh + 1) * 128],
                             Wh2c_st[:, mh, lc * 128:(lc + 1) * 128])
                pe_transpose(Wc2kT_sb[:, lc, mh * 128:(mh + 1) * 128],
                             Wc2k_st[:, mh, lc * 128:(lc + 1) * 128])

        # W_x2cont_slice^T chunks
        wstage = ctx.enter_context(tc.tile_pool(name="wstage", bufs=4))
        wxp = ctx.enter_context(tc.tile_pool(name="wxp", bufs=2))
        Wx2cT_sb = singles.tile([128, len(KCHUNKS), CONT], BF16, tag="Wx2cT_sb")
        for mh in range(2):
            st = wxp.tile([128, WSL], F32, tag="wx_st")
            nc.sync.dma_start(out=st, in_=Wx2cs_d[mh * 128:(mh + 1) * 128, :])
            for k, (ko, wsz) in enumerate(KCHUNKS):
                pe_transpose(Wx2cT_sb[:wsz, k, mh * 128:(mh + 1) * 128],
                             st[:, ko:ko + wsz])

        # partial x_cont over this core's slice (launch the AllReduce early)
        pctrl = ctx.enter_context(tc.tile_pool(name="pctrl", bufs=2, space="PSUM"))
        ps_xc = pctrl.tile([B, CONT], F32, tag="ps_ctrl")
        for k, (ko, wsz) in enumerate(KCHUNKS):
            nc.tensor.matmul(ps_xc, lhsT=xT_sb[:wsz, k, :], rhs=Wx2cT_sb[:wsz, k, :],
                             start=(k == 0), stop=(k == len(KCHUNKS) - 1))
        xc_part = singles.tile([B, CONT], F32, tag="xc_part")
        nc.vector.tensor_copy(xc_part, ps_xc)
        cc_in = dramp.tile([B, CONT], F32, tag="cc_in")
        nc.sync.dma_start(out=cc_in, in_=xc_part)
        cc_out = dramp.tile([BL, CONT], F32, tag="cc_out")
        nc.gpsimd.collective_compute(
            "ReduceScatter", ALU.add, replica_groups=REPL,
            ins=[cc_in[:, :]], outs=[cc_out[:, :]])
        xc_full = singles.tile([BL, CONT], F32, tag="xc_full")
        nc.sync.dma_start(out=xc_full, in_=cc_out)

        # ---------------- phase 1: controller (local batch of 16) ----------------
        xcont_sb = singles.tile([BL, CONT], F32, tag="xcont_sb")
        ctrl_sb = singles.tile([BL, CONT], F32, tag="ctrl_sb")
        keyv_sb = singles.tile([BL, D], F32, tag="keyv_sb")
        cb_all = singles.tile([128, BL], F32, tag="cb_all")
        ns_all = singles.tile([128, BL], F32, tag="ns_all")

        if True:
            nc.vector.tensor_add(xcont_sb, xc_full, bx2c_rep)

            # h_cont + control
            ps_hc = pctrl.tile([BL, CONT], F32, tag="ps_ctrl")
            for lc in range(2):
                nc.tensor.matmul(ps_hc, lhsT=hT_sb[:, lc, :], rhs=Wh2cT_sb[:, lc, :],
                                 start=(lc == 0), stop=(lc == 1))
            t1 = singles.tile([BL, CONT], F32, tag="ph1_t1")
            nc.vector.tensor_add(t1, ps_hc, xcont_sb)
            nc.vector.tensor_add(t1, t1, bh2c_rep)
            nc.vector.tensor_scalar_max(ctrl_sb, t1, 0.0)

            ctrlT_sb = singles.tile([128, 2, BL], F32, tag="ctrlT_sb")
            for lc in range(2):
                pe_transpose(ctrlT_sb[:, lc, :], ctrl_sb[:, lc * 128:(lc + 1) * 128])

            # key_v = tanh(control @ Wc2k^T + b)
            ps_kv = pctrl.tile([BL, D], F32, tag="ps_ctrl")
            for lc in range(2):
                nc.tensor.matmul(ps_kv, lhsT=ctrlT_sb[:, lc, :], rhs=Wc2kT_sb[:, lc, :],
                                 start=(lc == 0), stop=(lc == 1))
            kv_pre = singles.tile([BL, D], F32, tag="kv_pre")
            nc.vector.tensor_add(kv_pre, ps_kv, bc2k_rep)
            nc.scalar.activation(keyv_sb, kv_pre, AF.Tanh)

            # strength, c = strength * rsqrt(||key||^2), negstr
            str_scr = singles.tile([BL, CONT], F32, tag="str_scr")
            str_pre = singles.tile([BL, 1], F32, tag="str_pre")
            nc.vector.scalar_tensor_tensor(out=str_scr, in0=ctrl_sb, scalar=1.0,
                                           in1=wstr_rep, op0=ALU.mult, op1=ALU.mult,
                                           accum_out=str_pre)
            strength = singles.tile([BL, 1], F32, tag="strength")
            nc.vector.tensor_add(strength, str_pre, bc2s_rep)
            nc.vector.tensor_scalar_max(strength, strength, 0.0)
            nc.vector.tensor_scalar_add(strength, strength, 1.0)
            kn_scr = singles.tile([BL, D], F32, tag="kn_scr")
            kn2 = singles.tile([BL, 1], F32, tag="kn2")
            nc.scalar.activation(kn_scr, keyv_sb, AF.Square, accum_out=kn2)
            kn_rs = singles.tile([BL, 1], F32, tag="kn_rs")
            dve_rsqrt(kn_rs, kn2, singles, "knrs", BL, 1)
            pack = singles.tile([BL, 2], F32, tag="pack")
            nc.vector.tensor_mul(pack[:, 0:1], strength, kn_rs)
            nc.vector.tensor_scalar_mul(pack[:, 1:2], strength, -1.0)


            # per-sample exp scale/bias broadcast across partitions
            keyv_loc = keyv_sb
            xcont_loc = xcont_sb
            h_loc = h_sb
            scal_loc = pack
            for col, dst, tg in ((0, cb_all, "cbr"), (1, ns_all, "nsr")):
                ptc = ptr.tile([128, 128], F32, tag="tr_ps")
                nc.tensor.transpose(ptc[:1, :BL], scal_loc[:, col:col + 1],
                                    identity[:BL, :BL])
                row = singles.tile([1, BL], F32, tag=tg)
                nc.vector.tensor_copy(row, ptc[:1, :BL])
                nc.gpsimd.partition_broadcast(dst, row)


        # W_ih^T / W_hh^T (bf16, phase 3)
        WihT_sb = singles.tile([128, 4, 4 * LSTM], BF16, tag="WihT_sb")
        for gc in range(8):
            st = wstage.tile([128, D + CONT], F32, tag="wih_st")
            nc.sync.dma_start(out=st, in_=Wih_d[gc * 128:(gc + 1) * 128, :])
            for kc in range(4):
                pe_transpose(WihT_sb[:, kc, gc * 128:(gc + 1) * 128],
                             st[:, kc * 128:(kc + 1) * 128])
        WhhT_sb = singles.tile([128, 2, 4 * LSTM], BF16, tag="WhhT_sb")
        for gc in range(8):
            st = wstage.tile([128, LSTM], F32, tag="whh_st")
            nc.sync.dma_start(out=st, in_=Whh_d[gc * 128:(gc + 1) * 128, :])
            for lc in range(2):
                pe_transpose(WhhT_sb[:, lc, gc * 128:(gc + 1) * 128],
                             st[:, lc * 128:(lc + 1) * 128])
        WahT_sb = singles.tile([128, 2, AH], BF16, tag="WahT_sb")
        WchT_sb = singles.tile([128, 2, CH], BF16, tag="WchT_sb")
        st_ah = singles.tile([AH, LSTM], F32, tag="st_ah")
        nc.sync.dma_start(out=st_ah, in_=Wah_d[:, :])
        st_ch = singles.tile([CH, LSTM], F32, tag="st_ch")
        nc.sync.dma_start(out=st_ch, in_=Wch_d[:, :])
        for lc in range(2):
            pe_transpose(WahT_sb[:, lc, :], st_ah[:, lc * 128:(lc + 1) * 128])
            pe_transpose(WchT_sb[:, lc, :], st_ch[:, lc * 128:(lc + 1) * 128])
        # W_actor_slice^T (bf16) [128 ah, 1250]
        WactT_sb = singles.tile([128, WSL], BF16, tag="WactT_sb")
        for k, (ko, wsz) in enumerate(KCHUNKS):
            st = wstage.tile([128, AH], F32, tag="wact_st")
            nc.sync.dma_start(out=st[:wsz, :], in_=Wacts_d[ko:ko + wsz, :])
            pe_transpose(WactT_sb[:, ko:ko + wsz], st[:wsz, :])


        # ---------------- phase 2: memory read (per sample streaming) ----------------
        catt_all = singles.tile([BL, D], F32, tag="catt_all")
        keyv_dram = dramp.tile([BL, D], F32, tag="keyv_dram")
        nc.sync.dma_start(out=keyv_dram, in_=keyv_loc)

        featp = ctx.enter_context(tc.tile_pool(name="featp", bufs=4))
        keyrp = ctx.enter_context(tc.tile_pool(name="keyrp", bufs=3))
        scrp = ctx.enter_context(tc.tile_pool(name="scrp", bufs=2))
        smallp = ctx.enter_context(tc.tile_pool(name="smallp", bufs=4))
        tinyp = ctx.enter_context(tc.tile_pool(name="tinyp", bufs=2))
        with tc.tile_pool(name="pcand", bufs=2, space="PSUM") as pcand:
            for b in (range(BL) if phase_limit >= 2 else []):
                key_rep = keyrp.tile([128, D], F32, tag="key_rep")
                nc.sync.dma_start(out=key_rep, in_=_bcast_ap(keyv_dram[b], 128))
                cb = cb_all[:, b:b + 1]
                nsb = ns_all[:, b:b + 1]

                fsrc = feat_d[b].rearrange("(p j) d -> p j d", p=128)
                fhalves = []
                for hf in range(2):
                    fh = featp.tile([128, HT, D], F32, tag="fbh")
                    for g in range(2):
                        go = hf * HT + g * 8
                        nc.sync.dma_start(out=fh[:, g * 8:(g + 1) * 8, :],
                                          in_=fsrc[:, go:go + 8, :])
                    fhalves.append(fh)

                def fbt(t):
                    return fhalves[t // HT][:, t % HT, :]

                ps_cd = pcand.tile([1, D + 1], F32, tag="ps_cd")
                for hf in range(2):
                    dots = smallp.tile([128, HT], F32, tag="dots")
                    nrm2 = smallp.tile([128, HT], F32, tag="nrm2")
                    for tl in range(HT):
                        t = hf * HT + tl
                        scr = scrp.tile([128, D], F32, tag="scr_dve")
                        nc.vector.scalar_tensor_tensor(
                            out=scr, in0=fbt(t), scalar=1.0, in1=key_rep,
                            op0=ALU.mult, op1=ALU.mult, accum_out=dots[:, tl:tl + 1])
                        if t in DVE_NORM_TILES:
                            scr2 = scrp.tile([128, D], F32, tag="scr_dve2")
                            nc.vector.scalar_tensor_tensor(
                                out=scr2, in0=fbt(t), scalar=1.0, in1=fbt(t),
                                op0=ALU.mult, op1=ALU.mult,
                                accum_out=nrm2[:, tl:tl + 1])
                        else:
                            scr3 = scrp.tile([128, D], F32, tag="sq_scr")
                            nc.scalar.activation(scr3, fbt(t), AF.Square,
                                                 accum_out=nrm2[:, tl:tl + 1])

                    rs = smallp.tile([128, HT], F32, tag="rs")
                    dve_rsqrt(rs, nrm2, smallp, "p2rs", 128, HT, iters=1)
                    u = smallp.tile([128, HT], F32, tag="u")
                    nc.vector.tensor_mul(u, dots, rs)
                    e = smallp.tile([128, HT], F32, tag="e")
                    if hf == 0:
                        esum_all = smallp.tile([128, 2], F32, tag="esum_all")
                    nc.scalar.activation(e, u, AF.Exp, scale=cb, bias=nsb,
                                         accum_out=esum_all[:, hf:hf + 1])

                    for tl in range(HT):
                        t = hf * HT + tl
                        nc.tensor.matmul(ps_cd[:, 0:D], lhsT=e[:, tl:tl + 1],
                                         rhs=fbt(t), start=(t == 0), stop=(t == NT - 1))
                esum_tot = smallp.tile([128, 1], F32, tag="esum_tot")
                nc.vector.tensor_reduce(out=esum_tot, in_=esum_all,
                                        axis=mybir.AxisListType.X, op=ALU.add)
                nc.tensor.matmul(ps_cd[:, D:D + 1], lhsT=esum_tot, rhs=ones_col,
                                 start=True, stop=True)
                rden = tinyp.tile([1, 1], F32, tag="rden")
                nc.vector.reciprocal(rden, ps_cd[:, D:D + 1])
                catt_tmp = tinyp.tile([1, D], F32, tag="catt_tmp")
                nc.vector.tensor_scalar_mul(catt_tmp, ps_cd[:, 0:D], rden)
                nc.sync.dma_start(out=catt_all[b:b + 1, :], in_=catt_tmp)

        if phase_limit < 3:
            nc.sync.dma_start(out=out_h[:, :], in_=h_loc)
            nc.sync.dma_start(out=out_c[:, :], in_=keyv_loc)
            nc.sync.dma_start(out=out_value[:, :], in_=scal_loc[:, 0:1])
            nc.sync.dma_start(out=out_logit[0:BL, 0:CONT], in_=xcont_loc)
            if phase_limit >= 2:
                nc.sync.dma_start(out=out_logit[0:BL, CONT:2 * CONT], in_=catt_all)

        if phase_limit >= 3:
            # ------------- phase 3: gate, LSTM, heads (local batch of 16) -------------
            g_scr = singles.tile([BL, CONT], F32, tag="g_scr")
            ga1 = singles.tile([BL, 1], F32, tag="ga1")
            ga2 = singles.tile([BL, 1], F32, tag="ga2")
            ga3 = singles.tile([BL, 1], F32, tag="ga3")
            gsum = singles.tile([BL, 1], F32, tag="gsum")
            nc.vector.scalar_tensor_tensor(out=g_scr, in0=catt_all, scalar=1.0,
                                           in1=watt_rep[:, 0:D], op0=ALU.mult,
                                           op1=ALU.mult, accum_out=ga1)
            nc.vector.scalar_tensor_tensor(out=g_scr, in0=h_loc, scalar=1.0,
                                           in1=watt_rep[:, D:D + LSTM], op0=ALU.mult,
                                           op1=ALU.mult, accum_out=ga2)
            nc.vector.scalar_tensor_tensor(out=g_scr, in0=xcont_loc, scalar=1.0,
                                           in1=watt_rep[:, D + LSTM:D + LSTM + CONT],
                                           op0=ALU.mult, op1=ALU.mult, accum_out=ga3)
            nc.vector.tensor_add(gsum, ga1, ga2)
            nc.vector.tensor_add(gsum, gsum, ga3)
            nc.vector.tensor_add(gsum, gsum, batt_rep)
            gate = singles.tile([BL, 1], F32, tag="gate")
            nc.scalar.activation(gate, gsum, AF.Sigmoid)
            att_sb = singles.tile([BL, D], F32, tag="att_sb")
            nc.vector.tensor_scalar_mul(att_sb, catt_all, gate)

            # LSTM (local)
            inpT_sb = singles.tile([128, 4, BL], BF16, tag="inpT_sb")
            hT_bf = singles.tile([128, 2, BL], BF16, tag="hT_bf")
            for lc in range(2):
                pe_transpose(inpT_sb[:, lc, :], att_sb[:, lc * 128:(lc + 1) * 128])
                pe_transpose(inpT_sb[:, 2 + lc, :], xcont_loc[:, lc * 128:(lc + 1) * 128])
                pe_transpose(hT_bf[:, lc, :], h_loc[:, lc * 128:(lc + 1) * 128])

            with tc.tile_pool(name="pg", bufs=1, space="PSUM") as pg, \
                 tc.tile_pool(name="pl", bufs=1, space="PSUM") as pl, \
                 tc.tile_pool(name="psm", bufs=1, space="PSUM") as psm:
                ps_g = pg.tile([BL, 4 * LSTM], F32, tag="ps_g")
                for nh in range(2):
                    sl = slice(nh * 512, (nh + 1) * 512)
                    for kc in range(4):
                        nc.tensor.matmul(ps_g[:, sl], lhsT=inpT_sb[:, kc, :],
                                         rhs=WihT_sb[:, kc, sl], start=(kc == 0), stop=False)
                    for lc in range(2):
                        nc.tensor.matmul(ps_g[:, sl], lhsT=hT_bf[:, lc, :],
                                         rhs=WhhT_sb[:, lc, sl], start=False, stop=(lc == 1))
                g_sb = singles.tile([BL, 4 * LSTM], F32, tag="g_sb")
                nc.vector.tensor_add(g_sb, ps_g, bih_rep)
                nc.vector.tensor_add(g_sb, g_sb, bhh_rep)

                i_s = singles.tile([BL, LSTM], F32, tag="i_s")
                f_s = singles.tile([BL, LSTM], F32, tag="f_s")
                gg_t = singles.tile([BL, LSTM], F32, tag="gg_t")
                o_s = singles.tile([BL, LSTM], F32, tag="o_s")
                nc.scalar.activation(i_s, g_sb[:, 0:256], AF.Sigmoid)
                nc.scalar.activation(f_s, g_sb[:, 256:512], AF.Sigmoid)
                nc.scalar.activation(gg_t, g_sb[:, 512:768], AF.Tanh)
                nc.scalar.activation(o_s, g_sb[:, 768:1024], AF.Sigmoid)

                m1 = singles.tile([BL, LSTM], F32, tag="m1")
                nc.vector.tensor_mul(m1, f_s, c_sb)
                m2 = singles.tile([BL, LSTM], F32, tag="m2")
                nc.vector.tensor_mul(m2, i_s, gg_t)
                c_out = singles.tile([BL, LSTM], F32, tag="c_out")
                nc.vector.tensor_add(c_out, m1, m2)
                tc_t = singles.tile([BL, LSTM], F32, tag="tc_t")
                nc.scalar.activation(tc_t, c_out, AF.Tanh)
                h_out = singles.tile([BL, LSTM], F32, tag="h_out")
                nc.vector.tensor_mul(h_out, o_s, tc_t)
                nc.sync.dma_start(out=out_h[:, :], in_=h_out)
                nc.sync.dma_start(out=out_c[:, :], in_=c_out)

                # actor hidden (local) then AllGather across cores
                houtT_sb = singles.tile([128, 2, BL], BF16, tag="houtT_sb")
                for lc in range(2):
                    pe_transpose(houtT_sb[:, lc, :], h_out[:, lc * 128:(lc + 1) * 128])
                ps_ah = psm.tile([BL, AH], F32, tag="ps_head")
                for lc in range(2):
                    nc.tensor.matmul(ps_ah, lhsT=houtT_sb[:, lc, :], rhs=WahT_sb[:, lc, :],
                                     start=(lc == 0), stop=(lc == 1))
                ha = singles.tile([BL, AH], F32, tag="ha")
                nc.vector.tensor_add(ha, ps_ah, bah_rep)
                nc.vector.tensor_scalar_max(ha, ha, 0.0)
                ag_in = dramp.tile([BL, AH], F32, tag="ag_in")
                nc.sync.dma_start(out=ag_in, in_=ha)
                ag_out = dramp.tile([B, AH], F32, tag="ag_out", addr_space="Shared")
                nc.gpsimd.collective_compute(
                    "AllGather", ALU.bypass, replica_groups=REPL,
                    ins=[ag_in[:, :]], outs=[ag_out[:, :]])
                ha_all = singles.tile([B, AH], F32, tag="ha_all")
                nc.sync.dma_start(out=ha_all, in_=ag_out)
                haT = singles.tile([128, B], BF16, tag="haT")
                pe_transpose(haT, ha_all)

                lsb = ctx.enter_context(tc.tile_pool(name="lsb", bufs=2))
                for so, slen in ASEGS:
                    sl = slice(so, so + slen)
                    ps_l = pl.tile([B, 512], F32, tag="ps_l")
                    nc.tensor.matmul(ps_l[:, :slen], lhsT=haT, rhs=WactT_sb[:, sl],
                                     start=True, stop=False)
                    nc.tensor.matmul(ps_l[:, :slen], lhsT=ones_row, rhs=bact_row[:, sl],
                                     start=False, stop=True)
                    l_sb = lsb.tile([B, 512], F32, tag="l_sb")
                    if so == 0:
                        nc.vector.tensor_copy(l_sb[:, :slen], ps_l[:, :slen])
                    else:
                        nc.scalar.copy(l_sb[:, :slen], ps_l[:, :slen])
                    nc.sync.dma_start(out=out_logit[:, sl], in_=l_sb[:, :slen])

                # critic head (local)
                ps_ch = psm.tile([BL, CH], F32, tag="ps_head")
                for lc in range(2):
                    nc.tensor.matmul(ps_ch, lhsT=houtT_sb[:, lc, :], rhs=WchT_sb[:, lc, :],
                                     start=(lc == 0), stop=(lc == 1))
                hc = singles.tile([BL, CH], F32, tag="hc")
                nc.vector.tensor_add(hc, ps_ch, bch_rep)
                nc.vector.tensor_scalar_max(hc, hc, 0.0)
                v_scr = singles.tile([BL, CH], F32, tag="v_scr")
                v_pre = singles.tile([BL, 1], F32, tag="v_pre")
                nc.vector.scalar_tensor_tensor(out=v_scr, in0=hc, scalar=1.0,
                                               in1=wcrit_rep, op0=ALU.mult, op1=ALU.mult,
                                               accum_out=v_pre)
                v_sb = singles.tile([BL, 1], F32, tag="v_sb")
                nc.vector.tensor_add(v_sb, v_pre, bcrit_rep)
                nc.sync.dma_start(out=out_value[:, :], in_=v_sb)

    if not nc.is_finalized():
        nc.finalize()
    return nc


_CACHE: dict = {}


def _get_nc() -> bass.Bass:
    if "nc" not in _CACHE:
        _CACHE["nc"] = build_nc()
    return _CACHE["nc"]


def make_in_maps(inputs: dict) -> list[dict]:
    ar = {k: np.ascontiguousarray(np.asarray(v, dtype=np.float32))
          for k, v in inputs.items()}
    in_maps = []
    for i in range(NCORES):
        m = {
            "x_slice": np.ascontiguousarray(ar["x"][:, i * WSL:(i + 1) * WSL]),
            "feat": np.ascontiguousarray(ar["feat"][i * BL:(i + 1) * BL]),
            "h_tm1": np.ascontiguousarray(ar["h_tm1"][i * BL:(i + 1) * BL]),
            "c_tm1": np.ascontiguousarray(ar["c_tm1"][i * BL:(i + 1) * BL]),
            "W_x2cont_slice": np.ascontiguousarray(ar["W_x2cont"][:, i * WSL:(i + 1) * WSL]),
            "W_actor_slice": np.ascontiguousarray(ar["W_actor"][i * WSL:(i + 1) * WSL]),
            "b_actor_slice": np.ascontiguousarray(ar["b_actor"][i * WSL:(i + 1) * WSL]),
        }
        for k in ("b_x2cont", "W_h2cont", "b_h2cont", "W_cont2key", "b_cont2key",
                  "W_cont2strength", "b_cont2strength", "W_att_gates", "b_att_gates",
                  "W_ih", "b_ih", "W_hh", "b_hh", "W_actor_hid", "b_actor_hid",
                  "W_critic_hid", "b_critic_hid", "W_critic", "b_critic"):
            m[k] = ar[k]
        in_maps.append(m)
    return in_maps


def run(inputs: dict, **kwargs):
    nc = _get_nc()
    return run_bass_kernel_spmd(nc, make_in_maps(inputs), core_ids=list(range(NCORES)),
                                **kwargs)


def kernel(**inputs):
    res = run(inputs)
    r = res.results
    logits = np.concatenate([r[i]["action_logit"] for i in range(NCORES)], axis=1)
    value = np.concatenate([r[i]["value"] for i in range(NCORES)], axis=0)
    h = np.concatenate([r[i]["h"] for i in range(NCORES)], axis=0)
    c = np.concatenate([r[i]["c"] for i in range(NCORES)], axis=0)
    return (logits, value, h, c)
